# revision 1
# baseline (speedup 1.0000x reference)
"""Trainium2 Bass kernel for nn_Decoder_14139032338987 (sparse_attention).

One decoder step: embedding -> 4-layer LSTM -> Gaussian local-window
attention -> output projection -> vocab logits.  B=128, H=1024, V=32000.

Distribution over 8 NeuronCores (B kept whole on every core):
  - LSTM: tensor-parallel.  Core r computes a 128-wide h-slice of all four
    gates, producing x'[:, slice_r]; slices are transposed on-chip and
    AllGathered between layers (the AG output [1024,128] is exactly the
    transposed activation the next layer's matmul needs as lhsT).
  - Attention: p-chain replicated (needs full `out`), window gather and
    softmax sharded over B (16 rows/core) with (l,b)-packed partition
    layout; ctx re-assembled with a 0/1 selector matmul, AllGathered.
  - o2 projection replicated; vocab projection sharded over V (4000/core).
Host does layout only: embedding row gather, weight transposes/casts,
slicing, and final concat of the per-core logit slices.

SPMD note: the program is identical on all cores; every core-dependent
access (this core's 16 batch rows) goes through indirect-DMA gathers whose
index tensors are per-core host constants.
"""

import numpy as np

H = 1024
V = 32000
NL = 4
W = 10
B = 128
T = 532
L = 2 * W + 1  # 21
HALF = 512
STD2 = (W / 2.0) ** 2  # 25.0
NC = 8
HS = H // NC     # 128 h-slice per core
BS = B // NC     # 16 batch rows per core
VS = V // NC     # 4000 vocab rows per core
NG = (L * BS + 127) // 128  # 3 gather groups of (l,b) rows
ROWS = L * BS  # 336

_CACHE = {}


def _build(S_val: float):
    import concourse.bass as bass
    import concourse.mybir as mybir
    import concourse.bacc as bacc
    import concourse.tile as tile

    dt = mybir.dt
    f32, bf16, i32 = dt.float32, dt.bfloat16, dt.int32
    AF = mybir.ActivationFunctionType
    OP = mybir.AluOpType
    AP = bass.AP
    IOA = bass.IndirectOffsetOnAxis

    nc = bacc.Bacc("TRN2", target_bir_lowering=False, debug=False,
                   enable_asserts=False, num_devices=NC)

    def din(name, shape, d):
        return nc.dram_tensor(name, shape, d, kind="ExternalInput").ap()

    # ---- inputs (per-core data supplied via in_maps) ----
    x0T = din("x0T", [H, B], bf16)
    h0T = din("h0T", [NL * H, B], bf16)
    c0s = din("c0s", [B, NL * HS], f32)
    wT = din("wT", [NL * 2 * H, 512], bf16)        # (l, src, k) tiles
    gbias = din("gbias", [1, NL * 512], f32)
    aW1T = din("aW1T", [H, HALF], f32)
    aW2r = din("aW2r", [128, HALF], f32)
    ab2 = din("ab2", [128, 1], f32)
    dW1T = din("dW1T", [2 * H, H], bf16)
    db1r = din("db1r", [1, H], f32)
    ab1r = din("ab1r", [1, HALF], f32)
    dW2T = din("dW2T", [H, VS], bf16)
    db2r = din("db2r", [1, VS], f32)
    encs = din("encs", [T * BS, H], f32)
    identb = din("identb", [128, 128], bf16)
    identf = din("identf", [128, 128], f32)
    iotaL = din("iotaL", [BS, L], f32)
    iotaRow = din("iotaRow", [128, NG], f32)
    clampR = din("clampR", [128, 1], f32)
    repMc = din("repMc", [128, 128], f32)
    selMfc = din("selMfc", [128, BS], f32)
    maskCc = din("maskCc", [128, 24], f32)
    lmapc = din("lmapc", [24, NG * 128], f32)
    selM = din("selM", [128, BS], bf16)
    idxc = din("idxc", [128, 2], i32)   # col0: r*16+p%16 ; col1[0:16]: r*16+p

    yout = nc.dram_tensor("yout", [B, VS], f32, kind="ExternalOutput").ap()

    RG = [list(range(NC))]

    with tile.TileContext(nc) as tc:
        with tc.tile_pool(name="const", bufs=1) as cp, \
             tc.tile_pool(name="dw2p", bufs=1) as dw2p, \
             tc.tile_pool(name="dram", bufs=1, space="DRAM") as dp, \
             tc.tile_pool(name="work", bufs=1) as wk, \
             tc.tile_pool(name="ps_tr", bufs=2, space="PSUM") as ps_tr:

            # ---- persistent weight loads (start immediately, overlap all) ----
            dw2_sb = dw2p.tile([128, 8 * VS], bf16)
            for k in range(8):
                nc.sync.dma_start(out=dw2_sb[:, k * VS:(k + 1) * VS],
                                  in_=dW2T[k * 128:(k + 1) * 128, :])
            aw1_sb = cp.tile([128, 8 * HALF], f32)
            for k in range(8):
                nc.sync.dma_start(out=aw1_sb[:, k * HALF:(k + 1) * HALF],
                                  in_=aW1T[k * 128:(k + 1) * 128, :])
            idb = cp.tile([128, 128], bf16)
            nc.sync.dma_start(out=idb[:], in_=identb[:])
            idf = cp.tile([128, 128], f32)
            nc.sync.dma_start(out=idf[:], in_=identf[:])
            selM_sb = cp.tile([128, BS], bf16)
            nc.sync.dma_start(out=selM_sb[:], in_=selM[:])
            iotaL_sb = cp.tile([BS, L], f32)
            nc.sync.dma_start(out=iotaL_sb[:], in_=iotaL[:])
            ab1_sb = cp.tile([1, HALF], f32)
            nc.sync.dma_start(out=ab1_sb[:], in_=ab1r[:])
            ones_f = cp.tile([1, 128], f32)
            nc.vector.memset(ones_f[:], 1.0)
            iotaR_sb = cp.tile([128, NG], f32)
            nc.sync.dma_start(out=iotaR_sb[:], in_=iotaRow[:])
            clamp_sb = cp.tile([128, 1], f32)
            nc.sync.dma_start(out=clamp_sb[:], in_=clampR[:])
            repM_sb = cp.tile([128, 128], f32)
            nc.sync.dma_start(out=repM_sb[:], in_=repMc[:])
            selMf_sb = cp.tile([128, BS], f32)
            nc.sync.dma_start(out=selMf_sb[:], in_=selMfc[:])
            maskC_sb = cp.tile([128, 24], f32)
            nc.sync.dma_start(out=maskC_sb[:], in_=maskCc[:])
            lmap_sb = cp.tile([24, NG * 128], f32)
            nc.sync.dma_start(out=lmap_sb[:], in_=lmapc[:])
            idxc_sb = cp.tile([128, 2], i32)
            nc.sync.dma_start(out=idxc_sb[:], in_=idxc[:])
            gb_sb = cp.tile([1, NL * 512], f32)
            nc.sync.dma_start(out=gb_sb[:], in_=gbias[:])
            aw2_sb = cp.tile([128, HALF], f32)
            nc.sync.dma_start(out=aw2_sb[:], in_=aW2r[:])
            ab2_sb = cp.tile([128, 1], f32)
            nc.sync.dma_start(out=ab2_sb[:], in_=ab2[:])

            def pbc(ap, n):
                # [1, F] -> [n, F] partition broadcast
                b = ap.partition_broadcast(n)
                return b.rearrange("p a f -> p (a f)") if b.ndim == 3 else b

            # ============================ LSTM ============================
            outT_sb = wk.tile([128, H], f32)      # final hidden, transposed
            outTb_sb = wk.tile([128, H], bf16)
            ago3 = dp.tile([H, 256], f32, name="ago3", addr_space="Shared")
            out_bh = dp.tile([B, H], f32, name="out_bh")

            with tc.tile_pool(name="lstmw", bufs=1) as lw, \
                 tc.tile_pool(name="lstm_work", bufs=1) as lwk, \
                 tc.tile_pool(name="ps_g", bufs=2, space="PSUM") as ps_g:
                w_sb = lw.tile([128, NL * 2 * 8 * 512], bf16)
                nc.sync.dma_start(
                    out=w_sb[:].rearrange("p (m c) -> p m c", c=512),
                    in_=wT[:].rearrange("(m p) c -> p m c", p=128))
                c0_sb = lwk.tile([B, NL * HS], f32)
                nc.sync.dma_start(out=c0_sb[:], in_=c0s[:])
                h0T_sb = lwk.tile([128, NL * 8 * 128], bf16)
                nc.sync.dma_start(
                    out=h0T_sb[:].rearrange("p (m b) -> p m b", b=128),
                    in_=h0T[:].rearrange("(m p) b -> p m b", p=128))
                xT_sb = lwk.tile([128, H], bf16, tag="xT", bufs=2)
                nc.sync.dma_start(
                    out=xT_sb[:].rearrange("p (k b) -> p k b", b=128),
                    in_=x0T[:].rearrange("(k p) b -> p k b", p=128))

                for l in range(NL):
                    g_ps = ps_g.tile([128, 512], f32, tag="g")
                    mm = 0
                    for src in range(2):
                        for k in range(8):
                            lhsT = (xT_sb[:, k * 128:(k + 1) * 128] if src == 0
                                    else h0T_sb[:, (l * 8 + k) * 128:(l * 8 + k + 1) * 128])
                            nc.tensor.matmul(
                                out=g_ps[:],
                                lhsT=lhsT,
                                rhs=w_sb[:, ((l * 2 + src) * 8 + k) * 512:
                                         ((l * 2 + src) * 8 + k + 1) * 512],
                                start=(mm == 0), stop=False)
                            mm += 1
                    nc.tensor.matmul(out=g_ps[:], lhsT=ones_f[:],
                                     rhs=gb_sb[:, l * 512:(l + 1) * 512],
                                     start=False, stop=True)
                    i_s = lwk.tile([128, 128], f32, tag="i_s")
                    f_s = lwk.tile([128, 128], f32, tag="f_s")
                    g_t = lwk.tile([128, 128], f32, tag="g_t")
                    o_s = lwk.tile([128, 128], f32, tag="o_s")
                    nc.scalar.activation(out=i_s[:], in_=g_ps[:, 0:128], func=AF.Sigmoid)
                    nc.scalar.activation(out=f_s[:], in_=g_ps[:, 128:256], func=AF.Sigmoid)
                    nc.scalar.activation(out=g_t[:], in_=g_ps[:, 256:384], func=AF.Tanh)
                    nc.scalar.activation(out=o_s[:], in_=g_ps[:, 384:512], func=AF.Sigmoid)
                    cnew = lwk.tile([128, 128], f32, tag="cnew")
                    nc.vector.tensor_mul(out=cnew[:], in0=f_s[:],
                                         in1=c0_sb[:, l * HS:(l + 1) * HS])
                    ig = lwk.tile([128, 128], f32, tag="ig")
                    nc.vector.tensor_mul(out=ig[:], in0=i_s[:], in1=g_t[:])
                    nc.vector.tensor_add(out=cnew[:], in0=cnew[:], in1=ig[:])
                    tc_t = lwk.tile([128, 128], f32, tag="tc_t")
                    nc.scalar.activation(out=tc_t[:], in_=cnew[:], func=AF.Tanh)
                    xf = lwk.tile([128, 128], f32, tag="xf")
                    nc.vector.tensor_mul(out=xf[:], in0=o_s[:], in1=tc_t[:])

                    if l < NL - 1:
                        xb = lwk.tile([128, 128], bf16, tag="xb")
                        nc.vector.tensor_copy(out=xb[:], in_=xf[:])
                        tr_ps = ps_g.tile([128, 128], bf16, tag="tr")
                        nc.tensor.transpose(out=tr_ps[:], in_=xb[:], identity=idb[:])
                        xTs = lwk.tile([128, 128], bf16, tag="xTs")
                        nc.vector.tensor_copy(out=xTs[:], in_=tr_ps[:])
                        agi = dp.tile([128, 128], bf16, name=f"agi{l}", tag=f"agi{l}")
                        nc.sync.dma_start(out=agi[:], in_=xTs[:])
                        ago = dp.tile([H, 128], bf16, name=f"ago{l}", tag=f"ago{l}",
                                      addr_space="Shared")
                        nc.gpsimd.collective_compute(
                            "AllGather", OP.bypass, replica_groups=RG,
                            ins=[agi[:]], outs=[ago[:]])
                        xT_sb = lwk.tile([128, H], bf16, tag="xT", bufs=2)
                        nc.sync.dma_start(
                            out=xT_sb[:].rearrange("p (k b) -> p k b", b=128),
                            in_=ago[:].rearrange("(k p) b -> p k b", p=128))
                    else:
                        tr_ps = ps_g.tile([128, 128], f32, tag="tr")
                        nc.tensor.transpose(out=tr_ps[:], in_=xf[:], identity=idf[:])
                        pk = lwk.tile([128, 256], f32, tag="pk")
                        nc.vector.tensor_copy(out=pk[:, 0:128], in_=tr_ps[:])
                        nc.vector.tensor_copy(out=pk[:, 128:256], in_=xf[:])
                        agi3 = dp.tile([128, 256], f32, name="agi3")
                        nc.sync.dma_start(out=agi3[:], in_=pk[:])
                        nc.gpsimd.collective_compute(
                            "AllGather", OP.bypass, replica_groups=RG,
                            ins=[agi3[:]], outs=[ago3[:]])
                        nc.sync.dma_start(
                            out=outT_sb[:].rearrange("p (k b) -> p k b", b=128),
                            in_=ago3[:, 0:128].rearrange("(k p) b -> p k b", p=128))
                        nc.vector.tensor_copy(out=outTb_sb[:], in_=outT_sb[:])
                        # reshuffle to out[b, h] layout in DRAM for the
                        # attention score broadcast (local DMA, no core offset)
                        nc.sync.dma_start(
                            out=out_bh[:].rearrange("b (k f) -> b k f", f=128),
                            in_=AP(ago3[:].tensor, 128,
                                   [[256, 128], [128 * 256, 8], [1, 128]]))

            # ============================ p-chain ============================
            with tc.tile_pool(name="att", bufs=1) as at:
              with tc.tile_pool(name="ps_a", bufs=1, space="PSUM") as ps_a:
                pt_ps = ps_a.tile([128, HALF], f32, tag="pt")
                for k in range(8):
                    nc.tensor.matmul(out=pt_ps[:],
                                     lhsT=outT_sb[:, k * 128:(k + 1) * 128],
                                     rhs=aw1_sb[:, k * HALF:(k + 1) * HALF],
                                     start=(k == 0), stop=False)
                nc.tensor.matmul(out=pt_ps[:], lhsT=ones_f[:], rhs=ab1_sb[:],
                                 start=False, stop=True)
                pt = at.tile([128, HALF], f32)
                nc.scalar.activation(out=pt[:], in_=pt_ps[:], func=AF.Tanh)
                scr5 = at.tile([128, HALF], f32)
                z = at.tile([128, 1], f32)
                nc.vector.scalar_tensor_tensor(
                    out=scr5[:], in0=pt[:], scalar=1.0,
                    in1=aw2_sb[:], op0=OP.mult, op1=OP.mult,
                    accum_out=z[:])
                sg = at.tile([128, 1], f32)
                nc.scalar.activation(out=sg[:], in_=z[:], func=AF.Sigmoid,
                                     bias=ab2_sb[:])
                s_f = at.tile([128, 1], f32)       # p - W  (pre-round)
                nc.vector.tensor_scalar_mul(out=s_f[:], in0=sg[:], scalar1=float(S_val))
                r_ = at.tile([128, 1], f32)
                nc.vector.tensor_scalar_add(out=r_[:], in0=s_f[:], scalar1=0.5)
                # floor(r_) robust to the f32->i32 cast rounding mode:
                # f = cast(r_); if f > r_ then f -= 1
                ti = at.tile([128, 1], i32)
                nc.vector.tensor_copy(out=ti[:], in_=r_[:])
                tf = at.tile([128, 1], f32)
                nc.vector.tensor_copy(out=tf[:], in_=ti[:])
                cond = at.tile([128, 1], f32)
                nc.vector.tensor_tensor(out=cond[:], in0=tf[:], in1=r_[:],
                                        op=OP.is_gt)
                stf = at.tile([128, 1], f32)       # start (rounded, float)
                nc.vector.tensor_sub(out=stf[:], in0=tf[:], in1=cond[:])
                sti = at.tile([128, 1], i32)
                nc.vector.tensor_copy(out=sti[:], in_=stf[:])
                d0 = at.tile([128, 1], f32)        # start - p  (= stf - s_f - W)
                nc.vector.tensor_sub(out=d0[:], in0=stf[:], in1=s_f[:])
                nc.vector.tensor_scalar_add(out=d0[:], in0=d0[:], scalar1=-float(W))

                # ---- cross-partition replication via 0/1 matmuls (no DMA) ----
                # pk2: [start_f | start-p] per global-b partition
                pk2 = at.tile([128, 2], f32)
                nc.vector.tensor_copy(out=pk2[:, 0:1], in_=stf[:])
                nc.vector.tensor_copy(out=pk2[:, 1:2], in_=d0[:])
                # stf16/d016 for this core's 16 batch rows:
                g16_ps = ps_a.tile([BS, 2], f32, tag="scr", bufs=1)
                nc.tensor.matmul(out=g16_ps[:], lhsT=repM_sb[:, 0:BS],
                                 rhs=pk2[:], start=True, stop=True)
                g16 = at.tile([BS, 2], f32)
                nc.vector.tensor_copy(out=g16[:], in_=g16_ps[:])
                stf16 = g16[:, 0:1]
                d016 = g16[:, 1:2]
                # start_f replicated to all (l,b) rows:
                str_ps = ps_a.tile([128, 1], f32, tag="scr", bufs=1)
                nc.tensor.matmul(out=str_ps[:], lhsT=repM_sb[:],
                                 rhs=stf[:], start=True, stop=True)
                stf_rep = at.tile([128, 1], f32)
                nc.vector.tensor_copy(out=stf_rep[:], in_=str_ps[:])
                orep = at.tile([128, H], f32)
                nc.gpsimd.indirect_dma_start(
                    out=orep[:], out_offset=None, in_=out_bh[:],
                    in_offset=IOA(ap=idxc_sb[:, 0:1], axis=0))

                # ==================== gather + score ====================
                sel = [at.tile([128, H], f32, name=f"sel{g}", tag=f"sel{g}")
                       for g in range(NG)]
                sc_col = at.tile([128, NG], f32)
                nc.vector.memset(sc_col[:], 0.0)
                cnt = [128, 128, ROWS - 256]
                for g in range(NG):
                    idxf = at.tile([128, 1], f32, tag="idxf", bufs=3)
                    nc.vector.tensor_scalar_mul(out=idxf[:], in0=stf_rep[:],
                                                scalar1=float(BS))
                    nc.vector.tensor_add(out=idxf[:], in0=idxf[:],
                                         in1=iotaR_sb[:, g:g + 1])
                    nc.vector.tensor_tensor(out=idxf[:], in0=idxf[:],
                                            in1=clamp_sb[:], op=OP.min)
                    idx = at.tile([128, 1], i32, tag="idx", bufs=3)
                    nc.vector.tensor_copy(out=idx[:], in_=idxf[:])
                    nc.gpsimd.indirect_dma_start(
                        out=sel[g][0:cnt[g], :], out_offset=None,
                        in_=encs[:],
                        in_offset=IOA(ap=idx[0:cnt[g], :1], axis=0))
                    scrH = at.tile([128, H], f32, tag="scrH", bufs=1)
                    nc.vector.scalar_tensor_tensor(
                        out=scrH[0:cnt[g], :], in0=orep[0:cnt[g], :], scalar=1.0,
                        in1=sel[g][0:cnt[g], :], op0=OP.mult, op1=OP.mult,
                        accum_out=sc_col[0:cnt[g], g:g + 1])

                # -------- [16, 21] softmax block --------
                # sc16[b, l] = sc_col[(l%8)*16+b, l//8] via a selector matmul:
                # X[p, c] = sc_col[p, c//8] * maskC[p, c]; sc24 = selMf.T @ X
                X = at.tile([128, 24], f32)
                nc.vector.tensor_tensor(
                    out=X[:].rearrange("p (g li) -> p g li", g=NG),
                    in0=sc_col[:].unsqueeze(2).broadcast_to([128, NG, 8]),
                    in1=maskC_sb[:].rearrange("p (g li) -> p g li", g=NG),
                    op=OP.mult)
                sc_ps = ps_a.tile([BS, 24], f32, tag="scr", bufs=1)
                nc.tensor.matmul(out=sc_ps[:], lhsT=selMf_sb[:], rhs=X[:],
                                 start=True, stop=True)
                sc24 = at.tile([BS, 24], f32)
                nc.vector.tensor_copy(out=sc24[:], in_=sc_ps[:])
                sc16 = sc24[:, 0:L]

                pos = at.tile([BS, L], f32)
                nc.vector.tensor_scalar(out=pos[:], in0=iotaL_sb[:],
                                        scalar1=stf16, scalar2=None, op0=OP.add)
                v1 = at.tile([BS, L], f32)
                nc.vector.tensor_scalar(out=v1[:], in0=pos[:], scalar1=float(W),
                                        scalar2=None, op0=OP.is_ge)
                v2 = at.tile([BS, L], f32)
                nc.vector.tensor_scalar(out=v2[:], in0=pos[:],
                                        scalar1=float(S_val + W),
                                        scalar2=None, op0=OP.is_lt)
                nc.vector.tensor_mul(out=v1[:], in0=v1[:], in1=v2[:])
                sm = at.tile([BS, L], f32)
                nc.vector.tensor_scalar_add(out=sm[:], in0=sc16, scalar1=-1e-12)
                nc.vector.tensor_mul(out=sm[:], in0=sm[:], in1=v1[:])
                nc.vector.tensor_scalar_add(out=sm[:], in0=sm[:], scalar1=1e-12)
                mx = at.tile([BS, 1], f32)
                nc.vector.tensor_reduce(out=mx[:], in_=sm[:],
                                        axis=mybir.AxisListType.X, op=OP.max)
                nmx = at.tile([BS, 1], f32)
                nc.vector.tensor_scalar_mul(out=nmx[:], in0=mx[:], scalar1=-1.0)
                ex = at.tile([BS, L], f32)
                se = at.tile([BS, 1], f32)
                nc.scalar.activation(out=ex[:], in_=sm[:], func=AF.Exp,
                                     bias=nmx[:], accum_out=se[:])
                ri = at.tile([BS, 1], f32)
                nc.vector.reciprocal(out=ri[:], in_=se[:])
                aa = at.tile([BS, L], f32)
                nc.vector.tensor_scalar(out=aa[:], in0=ex[:], scalar1=ri[:],
                                        scalar2=None, op0=OP.mult)
                # gauss: pos - p = l + (start - p) = l + d016
                dd = at.tile([BS, L], f32)
                nc.vector.tensor_scalar(out=dd[:], in0=iotaL_sb[:],
                                        scalar1=d016, scalar2=None, op0=OP.add)
                d2 = at.tile([BS, L], f32)
                nc.vector.tensor_mul(out=d2[:], in0=dd[:], in1=dd[:])
                gs = at.tile([BS, L], f32)
                nc.scalar.activation(out=gs[:], in_=d2[:], func=AF.Exp,
                                     scale=-1.0 / (2.0 * STD2))
                nc.vector.tensor_mul(out=aa[:], in0=aa[:], in1=gs[:])
                # relayout a -> a-weighted selector Sa via PE:
                # aaT = aa.T (PE transpose), Sa_g = (Lmap_g.T @ aaT) * selMf
                aa24 = at.tile([BS, 24], f32)
                nc.vector.memset(aa24[:], 0.0)
                nc.vector.tensor_copy(out=aa24[:, 0:L], in_=aa[:])
                aaT_ps = ps_a.tile([24, BS], f32, tag="scr", bufs=1)
                nc.tensor.transpose(out=aaT_ps[:], in_=aa24[:],
                                    identity=idf[0:BS, 0:BS])
                aaT = at.tile([24, BS], f32)
                nc.vector.tensor_copy(out=aaT[:], in_=aaT_ps[:])

                # ==================== ctx ====================
                ctx_ps = ps_a.tile([BS, H], f32, tag="ctx")
                for g in range(NG):
                    sa_ps = ps_a.tile([128, BS], f32, tag="sa", bufs=1)
                    nc.tensor.matmul(out=sa_ps[:],
                                     lhsT=lmap_sb[:, g * 128:(g + 1) * 128],
                                     rhs=aaT[:], start=True, stop=True)
                    sa = at.tile([128, BS], bf16, tag="sab", bufs=3)
                    nc.vector.tensor_mul(out=sa[:], in0=sa_ps[:],
                                         in1=selMf_sb[:])
                    scd = at.tile([128, H], bf16, tag="scd", bufs=3)
                    nc.vector.tensor_copy(out=scd[0:cnt[g], :],
                                          in_=sel[g][0:cnt[g], :])
                    for n in range(2):
                        nc.tensor.matmul(
                            out=ctx_ps[:, n * 512:(n + 1) * 512],
                            lhsT=sa[0:cnt[g], :],
                            rhs=scd[0:cnt[g], n * 512:(n + 1) * 512],
                            start=(g == 0), stop=(g == NG - 1))
                ctxb = at.tile([BS, H], bf16)
                nc.vector.tensor_copy(out=ctxb[:], in_=ctx_ps[:])
              if True:
                ctxi = dp.tile([BS, H], bf16, name="ctxi")
                nc.sync.dma_start(out=ctxi[:], in_=ctxb[:])
                ctxo = dp.tile([B, H], bf16, name="ctxo", addr_space="Shared")
                nc.gpsimd.collective_compute(
                    "AllGather", OP.bypass, replica_groups=RG,
                    ins=[ctxi[:]], outs=[ctxo[:]])

                # ==================== o2 ====================
                ctx_sb = at.tile([B, H], bf16)
                nc.sync.dma_start(out=ctx_sb[:], in_=ctxo[:])
                ctxT = at.tile([128, H], bf16)
                for k in range(8):
                    trp = ps_tr.tile([128, 128], bf16, tag="tr2")
                    nc.tensor.transpose(out=trp[:],
                                        in_=ctx_sb[:, k * 128:(k + 1) * 128],
                                        identity=idb[:])
                    nc.vector.tensor_copy(out=ctxT[:, k * 128:(k + 1) * 128],
                                          in_=trp[:])
                o2b = at.tile([128, H], bf16)
                o2T = at.tile([128, H], bf16)
                with tc.tile_pool(name="dw1p", bufs=1) as dw1p, \
                     tc.tile_pool(name="ps_o2", bufs=1, space="PSUM") as ps_o2:
                    db1_sb = dw1p.tile([1, H], f32)
                    nc.sync.dma_start(out=db1_sb[:], in_=db1r[:])
                    dw1_sb = dw1p.tile([128, 16 * H], bf16)
                    nc.sync.dma_start(
                        out=dw1_sb[:].rearrange("p (m c) -> p m c", c=H),
                        in_=dW1T[:].rearrange("(m p) c -> p m c", p=128))
                    o2_ps = ps_o2.tile([128, H], f32, tag="o2")
                    for k in range(16):
                        lhsT = (ctxT[:, k * 128:(k + 1) * 128] if k < 8
                                else outTb_sb[:, (k - 8) * 128:(k - 7) * 128])
                        for n in range(2):
                            nc.tensor.matmul(
                                out=o2_ps[:, n * 512:(n + 1) * 512],
                                lhsT=lhsT,
                                rhs=dw1_sb[:, k * H + n * 512:k * H + (n + 1) * 512],
                                start=(k == 0), stop=False)
                    for n in range(2):
                        nc.tensor.matmul(out=o2_ps[:, n * 512:(n + 1) * 512],
                                         lhsT=ones_f[:],
                                         rhs=db1_sb[:, n * 512:(n + 1) * 512],
                                         start=False, stop=(n == 1))
                    nc.scalar.activation(out=o2b[:], in_=o2_ps[:], func=AF.Tanh)
                    for k in range(8):
                        trp = ps_tr.tile([128, 128], bf16, tag="tr2")
                        nc.tensor.transpose(out=trp[:],
                                            in_=o2b[:, k * 128:(k + 1) * 128],
                                            identity=idb[:])
                        nc.vector.tensor_copy(out=o2T[:, k * 128:(k + 1) * 128],
                                              in_=trp[:])

                # ==================== vocab ====================
                with tc.tile_pool(name="ps_y", bufs=3, space="PSUM") as ps_y, \
                     tc.tile_pool(name="ysb", bufs=3) as ysb:
                    db2_sb = ysb.tile([1, VS], f32, bufs=1)
                    nc.sync.dma_start(out=db2_sb[:], in_=db2r[:])
                    nch = (VS + 511) // 512
                    for n in range(nch):
                        cw = min(512, VS - n * 512)
                        y_ps = ps_y.tile([128, 512], f32, tag="y")
                        for k in range(8):
                            nc.tensor.matmul(
                                out=y_ps[:, 0:cw],
                                lhsT=o2T[:, k * 128:(k + 1) * 128],
                                rhs=dw2_sb[:, k * VS + n * 512:k * VS + n * 512 + cw],
                                start=(k == 0), stop=False)
                        nc.tensor.matmul(
                            out=y_ps[:, 0:cw], lhsT=ones_f[:],
                            rhs=db2_sb[:, n * 512:n * 512 + cw],
                            start=False, stop=True)
                        y_sb = ysb.tile([128, 512], f32, tag="ysb")
                        nc.vector.tensor_copy(out=y_sb[:, 0:cw], in_=y_ps[:, 0:cw])
                        nc.sync.dma_start(out=yout[:, n * 512:n * 512 + cw],
                                          in_=y_sb[:, 0:cw])

    nc.compile()
    return nc


def _prep_inputs(inputs):
    """Host-side layout: returns list of per-core in_maps."""
    import ml_dtypes
    bf16 = ml_dtypes.bfloat16

    enc = np.asarray(inputs["encoder_output"], np.float32)      # [T, B, H]
    h0 = np.asarray(inputs["h0"], np.float32)
    c0 = np.asarray(inputs["c0"], np.float32)
    emb = np.asarray(inputs["emb"], np.float32)
    Wih = np.asarray(inputs["Wih"], np.float32)
    Whh = np.asarray(inputs["Whh"], np.float32)
    bih = np.asarray(inputs["bih"], np.float32)
    bhh = np.asarray(inputs["bhh"], np.float32)
    aW1 = np.asarray(inputs["aW1"], np.float32)
    aW2 = np.asarray(inputs["aW2"], np.float32)
    ab2 = np.asarray(inputs["ab2"], np.float32)
    dW1 = np.asarray(inputs["dW1"], np.float32)
    db1 = np.asarray(inputs["db1"], np.float32)
    dW2 = np.asarray(inputs["dW2"], np.float32)
    db2 = np.asarray(inputs["db2"], np.float32)
    word = np.asarray(inputs["word"]).astype(np.int64)

    x0 = emb[word[0]]                                            # [B, H]
    x0T = np.ascontiguousarray(x0.T).astype(bf16)
    h0T = np.ascontiguousarray(h0.transpose(0, 2, 1)).reshape(NL * H, B).astype(bf16)

    ident_b = np.eye(128, dtype=np.float32).astype(bf16)
    ident_f = np.eye(128, dtype=np.float32)
    iotaL = np.tile(np.arange(L, dtype=np.float32).reshape(1, L), (BS, 1))
    selMat = np.zeros((128, BS), np.float32)
    for p in range(128):
        selMat[p, p % BS] = 1.0
    selMat = selMat.astype(bf16)
    iotaRow = np.zeros((128, NG), np.float32)
    for g in range(NG):
        for p in range(128):
            r = g * 128 + p
            iotaRow[p, g] = float(r if r < ROWS else 0)
    clampR = ((T - 1) * BS + (np.arange(128) % BS)).astype(np.float32).reshape(128, 1)
    selMf = selMat_f = np.zeros((128, BS), np.float32)
    for p in range(128):
        selMat_f[p, p % BS] = 1.0
    maskC = np.zeros((128, 24), np.float32)
    for p in range(128):
        for c in range(24):
            if p // BS == c % 8:
                maskC[p, c] = 1.0
    lmap = np.zeros((24, NG * 128), np.float32)
    for g in range(NG):
        for row in range(128):
            lmap[g * 8 + row // BS, g * 128 + row] = 1.0

    dW1T = np.ascontiguousarray(dW1.T).astype(bf16)              # [2H, H]
    aW1T = np.ascontiguousarray(aW1.T)                           # [H, HALF] f32
    aW2r = np.tile(aW2.reshape(1, HALF), (128, 1)).astype(np.float32)
    ab2r = np.tile(ab2.reshape(1, 1), (128, 1)).astype(np.float32)
    db1r = db1.reshape(1, H)
    ab1r = np.asarray(inputs["ab1"], np.float32).reshape(1, HALF)

    in_maps = []
    for r in range(NC):
        hs = slice(r * HS, (r + 1) * HS)
        rows = np.concatenate([np.arange(g * H + r * HS, g * H + (r + 1) * HS)
                               for g in range(4)])
        wT_l = []
        gb = np.zeros((NL, 512), np.float32)
        for l in range(NL):
            wT_l.append(np.ascontiguousarray(Wih[l][rows, :].T))  # [H, 512]
            wT_l.append(np.ascontiguousarray(Whh[l][rows, :].T))
            gb[l] = bih[l][rows] + bhh[l][rows]
        wT = np.concatenate(wT_l, axis=0).astype(bf16)           # [NL*2*H, 512]
        c0s = np.ascontiguousarray(
            np.stack([c0[l][:, hs] for l in range(NL)], axis=1).reshape(B, NL * HS))
        bs = slice(r * BS, (r + 1) * BS)
        encs = np.ascontiguousarray(enc[:, bs, :]).reshape(T * BS, H)
        vs = slice(r * VS, (r + 1) * VS)
        dW2T = np.ascontiguousarray(dW2[vs, :].T).astype(bf16)   # [H, VS]
        db2r_c = db2[vs].reshape(1, VS)
        idxc = np.zeros((128, 2), np.int32)
        idxc[:, 0] = r * BS + (np.arange(128) % BS)
        idxc[0:BS, 1] = r * BS + np.arange(BS)
        repM = np.zeros((128, 128), np.float32)
        for m in range(128):
            repM[r * BS + (m % BS), m] = 1.0
        in_maps.append({
            "x0T": np.ascontiguousarray(x0T),
            "h0T": h0T, "c0s": c0s, "wT": wT, "gbias": gb.reshape(1, NL * 512),
            "aW1T": aW1T, "aW2r": aW2r, "ab2": ab2r,
            "dW1T": dW1T, "db1r": db1r, "ab1r": ab1r, "dW2T": dW2T, "db2r": db2r_c,
            "encs": encs, "identb": ident_b, "identf": ident_f,
            "iotaL": iotaL, "iotaRow": iotaRow, "clampR": clampR,
            "selM": selMat, "idxc": idxc, "repMc": repM,
            "selMfc": selMf, "maskCc": maskC, "lmapc": lmap,
        })
    return in_maps


def kernel(**inputs):
    from concourse import bass_utils
    S_val = float(np.asarray(inputs["S"]))
    key = ("mod", S_val)
    if key not in _CACHE:
        _CACHE[key] = _build(S_val)
    nc = _CACHE[key]
    in_maps = _prep_inputs(inputs)
    res = bass_utils.run_bass_kernel_spmd(nc, in_maps, core_ids=list(range(NC)))
    y = np.concatenate([res.results[r]["yout"] for r in range(NC)], axis=1)
    return y.reshape(1, B, V).astype(np.float32)



# revision 7
# speedup vs baseline: 1.2053x; 1.2053x over previous
"""Trainium2 Bass kernel for nn_Decoder_14139032338987 (sparse_attention).

One decoder step: embedding -> 4-layer LSTM -> Gaussian local-window
attention -> output projection -> vocab logits.  B=128, H=1024, V=32000.

Distribution over 8 NeuronCores (B kept whole on every core):
  - LSTM: tensor-parallel.  Core r computes a 128-wide h-slice of all four
    gates, producing x'[:, slice_r]; slices are transposed on-chip and
    AllGathered between layers (the AG output [1024,128] is exactly the
    transposed activation the next layer's matmul needs as lhsT).
  - Attention: p-chain replicated (needs full `out`), window gather and
    softmax sharded over B (16 rows/core) with (l,b)-packed partition
    layout; ctx re-assembled with a 0/1 selector matmul, AllGathered.
  - o2 projection replicated; vocab projection sharded over V (4000/core).
Host does layout only: embedding row gather, weight transposes/casts,
slicing, final concat of the per-core logit slices, and the vocab bias add.

Scheduling notes (from trace analysis):
  - DMA issue order is critical-path order: LSTM inputs first (split per
    layer), aW1 next; dW1 triggers right before the out-AG, dW2 triggers
    after layer-1's load-back so the big loads never starve layer 0.
  - All bias adds are K=1 bf16 matmuls (ones row x bias row) -- fp32
    bias matmuls run in LOW_HIGH mode and cost ~6x more PE time.
  - Gate layout is [i,f,o,g] so one sigmoid covers i/f/o.
  - Per layer the h-half matmuls are issued before the x-half so they
    execute during the previous layer's AllGather gap.
  - The out AllGather carries only the f32 outT chunk (64KB); the
    replicated out-rows needed for scores are rebuilt on-chip via PE
    transposes + a 0/1 selector matmul.
  - Encoder windows are gathered in bf16 (2KB rows) and consumed directly
    by both the score reduction and the ctx matmul.
"""

import numpy as np

H = 1024
V = 32000
NL = 4
W = 10
B = 128
T = 532
L = 2 * W + 1  # 21
HALF = 512
STD2 = (W / 2.0) ** 2  # 25.0
NC = 8
HS = H // NC     # 128 h-slice per core
BS = B // NC     # 16 batch rows per core
VS = V // NC     # 4000 vocab rows per core
NG = (L * BS + 127) // 128  # 3 gather groups of (l,b) rows
ROWS = L * BS  # 336

_CACHE = {}


def _build(S_val: float):
    import concourse.bass as bass
    import concourse.mybir as mybir
    import concourse.bacc as bacc
    import concourse.tile as tile

    dt = mybir.dt
    f32, bf16, i32 = dt.float32, dt.bfloat16, dt.int32
    AF = mybir.ActivationFunctionType
    OP = mybir.AluOpType
    AP = bass.AP
    IOA = bass.IndirectOffsetOnAxis

    nc = bacc.Bacc("TRN2", target_bir_lowering=False, debug=False,
                   enable_asserts=False, num_devices=NC)

    def din(name, shape, d):
        return nc.dram_tensor(name, shape, d, kind="ExternalInput").ap()

    # ---- inputs (per-core data supplied via in_maps) ----
    x0T = din("x0T", [H, B], bf16)
    h0T = din("h0T", [NL * H, B], bf16)
    c0s = din("c0s", [B, NL * HS], f32)
    wT = din("wT", [NL * 2 * H, 512], bf16)        # (l, src, k) tiles
    gbias = din("gbias", [1, NL * 512], bf16)
    aW1T = din("aW1T", [H, HALF], f32)
    aW2r = din("aW2r", [128, HALF], f32)
    ab2 = din("ab2", [128, 1], f32)
    dW1T = din("dW1T", [2 * H, H], bf16)
    db1r = din("db1r", [1, H], bf16)
    ab1r = din("ab1r", [1, HALF], bf16)
    dW2T = din("dW2T", [H, VS], bf16)
    encs = din("encs", [T * BS, H], bf16)
    identb = din("identb", [128, 128], bf16)
    identf = din("identf", [128, 128], f32)
    iotaL = din("iotaL", [BS, L], f32)
    iotaRow = din("iotaRow", [128, NG], f32)
    clampR = din("clampR", [128, 1], f32)
    repMc = din("repMc", [128, 128], f32)
    selMfc = din("selMfc", [128, BS], f32)
    maskCc = din("maskCc", [128, 24], f32)
    lmapc = din("lmapc", [24, NG * 128], f32)

    yout = nc.dram_tensor("yout", [B, VS], f32, kind="ExternalOutput").ap()

    RG = [list(range(NC))]

    with tile.TileContext(nc) as tc:
        with tc.tile_pool(name="const", bufs=1) as cp, \
             tc.tile_pool(name="dw2p", bufs=1) as dw2p, \
             tc.tile_pool(name="dram", bufs=1, space="DRAM") as dp, \
             tc.tile_pool(name="work", bufs=1) as wk:

            # ---- critical-path loads first: LSTM inputs in layer order ----
            ones_b = cp.tile([1, 128], bf16)
            nc.vector.memset(ones_b[:], 1.0)
            idb = cp.tile([128, 128], bf16)
            nc.sync.dma_start(out=idb[:], in_=identb[:])
            idf = cp.tile([128, 128], f32)
            nc.sync.dma_start(out=idf[:], in_=identf[:])
            gb_sb = cp.tile([1, NL * 512], bf16)
            nc.sync.dma_start(out=gb_sb[:], in_=gbias[:])

            # LSTM weight/state tiles (own pool so SBUF frees before o2)
            with tc.tile_pool(name="lstmw", bufs=1) as lw, \
                 tc.tile_pool(name="lstm_work", bufs=1) as lwk, \
                 tc.tile_pool(name="ps_g", bufs=2, space="PSUM") as ps_g:
                xT_sb = lwk.tile([128, H], bf16, tag="xT", bufs=2)
                nc.sync.dma_start(
                    out=xT_sb[:].rearrange("p (k b) -> p k b", b=128),
                    in_=x0T[:].rearrange("(k p) b -> p k b", p=128))
                h0T_sb = lwk.tile([128, NL * 8 * 128], bf16)
                for hh in range(2):
                    nc.sync.dma_start(
                        out=h0T_sb[:, hh * 2048:(hh + 1) * 2048].rearrange(
                            "p (m b) -> p m b", b=128),
                        in_=h0T[hh * 2 * H:(hh + 1) * 2 * H, :].rearrange(
                            "(m p) b -> p m b", p=128))
                w_sb = lw.tile([128, NL * 2 * 8 * 512], bf16)
                c0_sb = lwk.tile([B, NL * HS], f32)
                for l in range(NL):
                    nc.sync.dma_start(
                        out=w_sb[:, l * 8192:(l + 1) * 8192].rearrange(
                            "p (m c) -> p m c", c=512),
                        in_=wT[l * 2 * H:(l + 1) * 2 * H, :].rearrange(
                            "(m p) c -> p m c", p=128))
                    if l == 0:
                        nc.sync.dma_start(out=c0_sb[:, 0:2 * HS],
                                          in_=c0s[:, 0:2 * HS])
                    elif l == 1:
                        nc.sync.dma_start(out=c0_sb[:, 2 * HS:],
                                          in_=c0s[:, 2 * HS:])

                # attention p-chain weights: needed right after the LSTM
                aw1_sb = cp.tile([128, 8 * HALF], f32)
                nc.sync.dma_start(
                    out=aw1_sb[:].rearrange("p (k c) -> p k c", c=HALF),
                    in_=aW1T[:].rearrange("(k p) c -> p k c", p=128))
                # small consts (cheap, after the big critical loads)
                iotaL_sb = cp.tile([BS, L], f32)
                nc.sync.dma_start(out=iotaL_sb[:], in_=iotaL[:])
                ab1_sb = cp.tile([1, HALF], bf16)
                nc.sync.dma_start(out=ab1_sb[:], in_=ab1r[:])
                iotaR_sb = cp.tile([128, NG], f32)
                nc.sync.dma_start(out=iotaR_sb[:], in_=iotaRow[:])
                clamp_sb = cp.tile([128, 1], f32)
                nc.sync.dma_start(out=clamp_sb[:], in_=clampR[:])
                repM_sb = cp.tile([128, 128], f32)
                nc.sync.dma_start(out=repM_sb[:], in_=repMc[:])
                selMf_sb = cp.tile([128, BS], f32)
                nc.sync.dma_start(out=selMf_sb[:], in_=selMfc[:])
                maskC_sb = cp.tile([128, 24], f32)
                nc.sync.dma_start(out=maskC_sb[:], in_=maskCc[:])
                lmap_sb = cp.tile([24, NG * 128], f32)
                nc.sync.dma_start(out=lmap_sb[:], in_=lmapc[:])
                aw2_sb = cp.tile([128, HALF], f32)
                nc.sync.dma_start(out=aw2_sb[:], in_=aW2r[:])
                ab2_sb = cp.tile([128, 1], f32)
                nc.sync.dma_start(out=ab2_sb[:], in_=ab2[:])
                db1_sb = cp.tile([1, H], bf16)
                nc.sync.dma_start(out=db1_sb[:], in_=db1r[:])

                # big deferred loads: dW2 streams during layers 2-3 +
                # attention (trigger placed mid-LSTM below)
                dw2_sb = dw2p.tile([128, 8 * VS], bf16)

                outT_sb = wk.tile([128, H], f32)      # final hidden, transposed
                outTb_sb = wk.tile([128, H], bf16)
                o2T = wk.tile([128, H], bf16)
                ago3 = dp.tile([H, 128], f32, name="ago3", addr_space="Shared")

                # ============================ LSTM ============================
                for l in range(NL):
                    g_ps = ps_g.tile([128, 512], f32, tag="g")
                    # h-half first: it has no dependence on the previous
                    # layer's AllGather, so it fills the AG gap on the PE.
                    for k in range(8):
                        nc.tensor.matmul(
                            out=g_ps[:],
                            lhsT=h0T_sb[:, (l * 8 + k) * 128:(l * 8 + k + 1) * 128],
                            rhs=w_sb[:, ((l * 2 + 1) * 8 + k) * 512:
                                     ((l * 2 + 1) * 8 + k + 1) * 512],
                            start=(k == 0), stop=False)
                    nc.tensor.matmul(out=g_ps[:], lhsT=ones_b[:],
                                     rhs=gb_sb[:, l * 512:(l + 1) * 512],
                                     start=False, stop=False)
                    for k in range(8):
                        nc.tensor.matmul(
                            out=g_ps[:],
                            lhsT=xT_sb[:, k * 128:(k + 1) * 128],
                            rhs=w_sb[:, ((l * 2) * 8 + k) * 512:
                                     ((l * 2) * 8 + k + 1) * 512],
                            start=False, stop=(k == 7))
                    # gates packed [i,f,o,g]: one sigmoid covers i/f/o
                    ifo = lwk.tile([128, 384], f32, tag="ifo")
                    nc.scalar.activation(out=ifo[:], in_=g_ps[:, 0:384],
                                         func=AF.Sigmoid)
                    g_t = lwk.tile([128, 128], f32, tag="g_t")
                    nc.scalar.activation(out=g_t[:], in_=g_ps[:, 384:512],
                                         func=AF.Tanh)
                    cnew = lwk.tile([128, 128], f32, tag="cnew")
                    nc.vector.tensor_mul(out=cnew[:], in0=ifo[:, 128:256],
                                         in1=c0_sb[:, l * HS:(l + 1) * HS])
                    ig = lwk.tile([128, 128], f32, tag="ig")
                    nc.vector.tensor_mul(out=ig[:], in0=ifo[:, 0:128], in1=g_t[:])
                    nc.vector.tensor_add(out=cnew[:], in0=cnew[:], in1=ig[:])
                    tc_t = lwk.tile([128, 128], f32, tag="tc_t")
                    nc.scalar.activation(out=tc_t[:], in_=cnew[:], func=AF.Tanh)
                    xf = lwk.tile([128, 128], f32, tag="xf")
                    nc.vector.tensor_mul(out=xf[:], in0=ifo[:, 256:384],
                                         in1=tc_t[:])

                    if l < NL - 1:
                        xb = lwk.tile([128, 128], bf16, tag="xb")
                        nc.vector.tensor_copy(out=xb[:], in_=xf[:])
                        tr_ps = ps_g.tile([128, 128], bf16, tag="tr")
                        nc.tensor.transpose(out=tr_ps[:], in_=xb[:], identity=idb[:])
                        xTs = lwk.tile([128, 128], bf16, tag="xTs")
                        nc.vector.tensor_copy(out=xTs[:], in_=tr_ps[:])
                        agi = dp.tile([128, 128], bf16, name=f"agi{l}", tag=f"agi{l}")
                        nc.sync.dma_start(out=agi[:], in_=xTs[:])
                        ago = dp.tile([H, 128], bf16, name=f"ago{l}", tag=f"ago{l}",
                                      addr_space="Shared")
                        nc.gpsimd.collective_compute(
                            "AllGather", OP.bypass, replica_groups=RG,
                            ins=[agi[:]], outs=[ago[:]])
                        xT_sb = lwk.tile([128, H], bf16, tag="xT", bufs=2)
                        nc.sync.dma_start(
                            out=xT_sb[:].rearrange("p (k b) -> p k b", b=128),
                            in_=ago[:].rearrange("(k p) b -> p k b", p=128))
                        if l == 1:
                            # dW2 (8MB) trigger sits after layer-1's
                            # load-back: transfer runs during layers 2-3 +
                            # attention, never contending with LSTM loads.
                            nc.sync.dma_start(
                                out=dw2_sb[:].rearrange("p (k c) -> p k c", c=VS),
                                in_=dW2T[:].rearrange("(k p) c -> p k c", p=128))
                    else:
                        tr_ps = ps_g.tile([128, 128], f32, tag="trf")
                        nc.tensor.transpose(out=tr_ps[:], in_=xf[:], identity=idf[:])
                        agi3 = dp.tile([128, 128], f32, name="agi3")
                        xTf = lwk.tile([128, 128], f32, tag="xTf")
                        nc.vector.tensor_copy(out=xTf[:], in_=tr_ps[:])
                        nc.sync.dma_start(out=agi3[:], in_=xTf[:])
                        nc.gpsimd.collective_compute(
                            "AllGather", OP.bypass, replica_groups=RG,
                            ins=[agi3[:]], outs=[ago3[:]])

            # ---- dW1: trigger before the out-AG load-back; the 4MB
            # transfer overlaps the AllGather itself.
            with tc.tile_pool(name="dw1p", bufs=1) as dw1p:
                dw1_sb = dw1p.tile([128, 16 * H], bf16)
                nc.sync.dma_start(
                    out=dw1_sb[:].rearrange("p (m c) -> p m c", c=H),
                    in_=dW1T[:].rearrange("(m p) c -> p m c", p=128))
                nc.sync.dma_start(
                    out=outT_sb[:].rearrange("p (k b) -> p k b", b=128),
                    in_=ago3[:].rearrange("(k p) b -> p k b", p=128))
                nc.vector.tensor_copy(out=outTb_sb[:], in_=outT_sb[:])

                # ============================ p-chain ============================
                with tc.tile_pool(name="att", bufs=1) as at, \
                     tc.tile_pool(name="ps_a", bufs=1, space="PSUM") as ps_a:
                    pt_ps = ps_a.tile([128, HALF], f32, tag="pt")
                    for k in range(8):
                        nc.tensor.matmul(out=pt_ps[:],
                                         lhsT=outT_sb[:, k * 128:(k + 1) * 128],
                                         rhs=aw1_sb[:, k * HALF:(k + 1) * HALF],
                                         start=(k == 0), stop=False)
                    nc.tensor.matmul(out=pt_ps[:], lhsT=ones_b[:], rhs=ab1_sb[:],
                                     start=False, stop=True)
                    pt = at.tile([128, HALF], f32)
                    nc.scalar.activation(out=pt[:], in_=pt_ps[:], func=AF.Tanh)
                    scr5 = at.tile([128, HALF], f32)
                    z = at.tile([128, 1], f32)
                    nc.vector.scalar_tensor_tensor(
                        out=scr5[:], in0=pt[:], scalar=1.0,
                        in1=aw2_sb[:], op0=OP.mult, op1=OP.mult,
                        accum_out=z[:])
                    sg = at.tile([128, 1], f32)
                    nc.scalar.activation(out=sg[:], in_=z[:], func=AF.Sigmoid,
                                         bias=ab2_sb[:])
                    s_f = at.tile([128, 1], f32)       # p - W  (pre-round)
                    nc.vector.tensor_scalar_mul(out=s_f[:], in0=sg[:],
                                                scalar1=float(S_val))
                    r_ = at.tile([128, 1], f32)
                    nc.vector.tensor_scalar_add(out=r_[:], in0=s_f[:], scalar1=0.5)
                    # floor(r_) robust to the f32->i32 cast rounding mode:
                    # f = cast(r_); if f > r_ then f -= 1
                    ti = at.tile([128, 1], i32)
                    nc.vector.tensor_copy(out=ti[:], in_=r_[:])
                    tf = at.tile([128, 1], f32)
                    nc.vector.tensor_copy(out=tf[:], in_=ti[:])
                    cond = at.tile([128, 1], f32)
                    nc.vector.tensor_tensor(out=cond[:], in0=tf[:], in1=r_[:],
                                            op=OP.is_gt)
                    stf = at.tile([128, 1], f32)       # start (rounded, float)
                    nc.vector.tensor_sub(out=stf[:], in0=tf[:], in1=cond[:])
                    d0 = at.tile([128, 1], f32)        # start - p  (= stf - s_f - W)
                    nc.vector.tensor_sub(out=d0[:], in0=stf[:], in1=s_f[:])
                    nc.vector.tensor_scalar_add(out=d0[:], in0=d0[:],
                                                scalar1=-float(W))

                    # ---- on-chip replicated out rows (this core's 16 b) ----
                    # rebuild out[b,h] via 8 full PE transposes of outT
                    # chunks, then orep = repM.T @ out replicates this
                    # core's 16 rows to the 128 (l,b) partition rows
                    # (repM is a per-core host constant, program is SPMD).
                    out_full = at.tile([128, H], f32)
                    for k in range(8):
                        tf_ps = ps_a.tile([128, 128], f32, tag="t16", bufs=2)
                        nc.tensor.transpose(
                            out=tf_ps[:],
                            in_=outT_sb[:, k * 128:(k + 1) * 128],
                            identity=idf[:])
                        nc.vector.tensor_copy(out=out_full[:, k * 128:(k + 1) * 128],
                                              in_=tf_ps[:])
                    orep_ps = ps_a.tile([128, H], f32, tag="orep")
                    for n in range(2):
                        nc.tensor.matmul(out=orep_ps[:, n * 512:(n + 1) * 512],
                                         lhsT=repM_sb[:],
                                         rhs=out_full[:, n * 512:(n + 1) * 512],
                                         start=True, stop=True)
                    orep = at.tile([128, H], f32)
                    nc.vector.tensor_copy(out=orep[:, 0:512], in_=orep_ps[:, 0:512])
                    nc.vector.tensor_copy(out=orep[:, 512:1024],
                                          in_=orep_ps[:, 512:1024])

                    # ---- cross-partition replication via 0/1 matmuls ----
                    pk2 = at.tile([128, 2], f32)
                    nc.vector.tensor_copy(out=pk2[:, 0:1], in_=stf[:])
                    nc.vector.tensor_copy(out=pk2[:, 1:2], in_=d0[:])
                    g16_ps = ps_a.tile([BS, 2], f32, tag="scr", bufs=1)
                    nc.tensor.matmul(out=g16_ps[:], lhsT=repM_sb[:, 0:BS],
                                     rhs=pk2[:], start=True, stop=True)
                    g16 = at.tile([BS, 2], f32)
                    nc.vector.tensor_copy(out=g16[:], in_=g16_ps[:])
                    stf16 = g16[:, 0:1]
                    d016 = g16[:, 1:2]
                    str_ps = ps_a.tile([128, 1], f32, tag="scr", bufs=1)
                    nc.tensor.matmul(out=str_ps[:], lhsT=repM_sb[:],
                                     rhs=stf[:], start=True, stop=True)
                    stf_rep = at.tile([128, 1], f32)
                    nc.vector.tensor_copy(out=stf_rep[:], in_=str_ps[:])

                    # ==================== gather ====================
                    sel = [at.tile([128, H], bf16, name=f"sel{g}", tag=f"sel{g}")
                           for g in range(NG)]
                    cnt = [128, 128, ROWS - 256]
                    idxs = []
                    for g in range(NG):
                        idxf = at.tile([128, 1], f32, tag=f"idxf{g}")
                        nc.vector.tensor_scalar_mul(out=idxf[:], in0=stf_rep[:],
                                                    scalar1=float(BS))
                        nc.vector.tensor_add(out=idxf[:], in0=idxf[:],
                                             in1=iotaR_sb[:, g:g + 1])
                        nc.vector.tensor_tensor(out=idxf[:], in0=idxf[:],
                                                in1=clamp_sb[:], op=OP.min)
                        idx = at.tile([128, 1], i32, tag=f"idx{g}")
                        nc.vector.tensor_copy(out=idx[:], in_=idxf[:])
                        idxs.append(idx)
                    for g in range(NG):
                        nc.gpsimd.indirect_dma_start(
                            out=sel[g][0:cnt[g], :], out_offset=None,
                            in_=encs[:],
                            in_offset=IOA(ap=idxs[g][0:cnt[g], :1], axis=0))

                    # ---- window masks + gaussian (overlap the gathers) ----
                    pos = at.tile([BS, L], f32)
                    nc.vector.tensor_scalar(out=pos[:], in0=iotaL_sb[:],
                                            scalar1=stf16, scalar2=None, op0=OP.add)
                    v1 = at.tile([BS, L], f32)
                    nc.vector.tensor_scalar(out=v1[:], in0=pos[:], scalar1=float(W),
                                            scalar2=None, op0=OP.is_ge)
                    v2 = at.tile([BS, L], f32)
                    nc.vector.tensor_scalar(out=v2[:], in0=pos[:],
                                            scalar1=float(S_val + W),
                                            scalar2=None, op0=OP.is_lt)
                    nc.vector.tensor_mul(out=v1[:], in0=v1[:], in1=v2[:])
                    dd = at.tile([BS, L], f32)
                    nc.vector.tensor_scalar(out=dd[:], in0=iotaL_sb[:],
                                            scalar1=d016, scalar2=None, op0=OP.add)
                    d2 = at.tile([BS, L], f32)
                    nc.vector.tensor_mul(out=d2[:], in0=dd[:], in1=dd[:])
                    gs = at.tile([BS, L], f32)
                    nc.scalar.activation(out=gs[:], in_=d2[:], func=AF.Exp,
                                         scale=-1.0 / (2.0 * STD2))

                    # ==================== scores ====================
                    sc_col = at.tile([128, NG], f32)
                    nc.vector.memset(sc_col[:], 0.0)
                    scrH = at.tile([128, H], f32)
                    for g in range(NG):
                        nc.vector.scalar_tensor_tensor(
                            out=scrH[0:cnt[g], :], in0=orep[0:cnt[g], :], scalar=1.0,
                            in1=sel[g][0:cnt[g], :], op0=OP.mult, op1=OP.mult,
                            accum_out=sc_col[0:cnt[g], g:g + 1])

                    # -------- [16, 21] softmax block --------
                    X = at.tile([128, 24], f32)
                    nc.vector.tensor_tensor(
                        out=X[:].rearrange("p (g li) -> p g li", g=NG),
                        in0=sc_col[:].unsqueeze(2).broadcast_to([128, NG, 8]),
                        in1=maskC_sb[:].rearrange("p (g li) -> p g li", g=NG),
                        op=OP.mult)
                    sc_ps = ps_a.tile([BS, 24], f32, tag="scr", bufs=1)
                    nc.tensor.matmul(out=sc_ps[:], lhsT=selMf_sb[:], rhs=X[:],
                                     start=True, stop=True)
                    sc24 = at.tile([BS, 24], f32)
                    nc.vector.tensor_copy(out=sc24[:], in_=sc_ps[:])
                    sc16 = sc24[:, 0:L]

                    sm = at.tile([BS, L], f32)
                    nc.vector.tensor_scalar_add(out=sm[:], in0=sc16, scalar1=-1e-12)
                    nc.vector.tensor_mul(out=sm[:], in0=sm[:], in1=v1[:])
                    nc.vector.tensor_scalar_add(out=sm[:], in0=sm[:], scalar1=1e-12)
                    mx = at.tile([BS, 1], f32)
                    nc.vector.tensor_reduce(out=mx[:], in_=sm[:],
                                            axis=mybir.AxisListType.X, op=OP.max)
                    nmx = at.tile([BS, 1], f32)
                    nc.vector.tensor_scalar_mul(out=nmx[:], in0=mx[:], scalar1=-1.0)
                    ex = at.tile([BS, L], f32)
                    se = at.tile([BS, 1], f32)
                    nc.scalar.activation(out=ex[:], in_=sm[:], func=AF.Exp,
                                         bias=nmx[:], accum_out=se[:])
                    ri = at.tile([BS, 1], f32)
                    nc.vector.reciprocal(out=ri[:], in_=se[:])
                    aa = at.tile([BS, L], f32)
                    nc.vector.tensor_scalar(out=aa[:], in0=ex[:], scalar1=ri[:],
                                            scalar2=None, op0=OP.mult)
                    nc.vector.tensor_mul(out=aa[:], in0=aa[:], in1=gs[:])
                    # relayout a -> a-weighted selector Sa via PE
                    aa24 = at.tile([BS, 24], f32)
                    nc.vector.memset(aa24[:], 0.0)
                    nc.vector.tensor_copy(out=aa24[:, 0:L], in_=aa[:])
                    aaT_ps = ps_a.tile([24, BS], f32, tag="scr", bufs=1)
                    nc.tensor.transpose(out=aaT_ps[:], in_=aa24[:],
                                        identity=idf[0:BS, 0:BS])
                    aaT = at.tile([24, BS], f32)
                    nc.vector.tensor_copy(out=aaT[:], in_=aaT_ps[:])

                    # ==================== ctx ====================
                    ctx_ps = ps_a.tile([BS, H], f32, tag="ctx")
                    for g in range(NG):
                        sa_ps = ps_a.tile([128, BS], f32, tag="scr", bufs=1)
                        nc.tensor.matmul(out=sa_ps[:],
                                         lhsT=lmap_sb[:, g * 128:(g + 1) * 128],
                                         rhs=aaT[:], start=True, stop=True)
                        sa = at.tile([128, BS], bf16, tag="sab", bufs=3)
                        nc.vector.tensor_mul(out=sa[:], in0=sa_ps[:],
                                             in1=selMf_sb[:])
                        for n in range(2):
                            nc.tensor.matmul(
                                out=ctx_ps[:, n * 512:(n + 1) * 512],
                                lhsT=sa[0:cnt[g], :],
                                rhs=sel[g][0:cnt[g], n * 512:(n + 1) * 512],
                                start=(g == 0), stop=(g == NG - 1))
                    ctxb = at.tile([BS, H], bf16)
                    nc.vector.tensor_copy(out=ctxb[:], in_=ctx_ps[:])
                    ctxi = dp.tile([BS, H], bf16, name="ctxi")
                    nc.sync.dma_start(out=ctxi[:], in_=ctxb[:])
                ctxo = dp.tile([B, H], bf16, name="ctxo", addr_space="Shared")
                nc.gpsimd.collective_compute(
                    "AllGather", OP.bypass, replica_groups=RG,
                    ins=[ctxi[:]], outs=[ctxo[:]])

                # ==================== o2 ====================
                o2b = wk.tile([128, H], bf16)
                with tc.tile_pool(name="o2w", bufs=1) as o2w, \
                     tc.tile_pool(name="ps_o2", bufs=1, space="PSUM") as ps_o2, \
                     tc.tile_pool(name="ps_tr", bufs=2, space="PSUM") as ps_tr:
                    o2_ps = ps_o2.tile([128, H], f32, tag="o2")
                    # out-half first: runs during the ctx AllGather
                    for k in range(8):
                        for n in range(2):
                            nc.tensor.matmul(
                                out=o2_ps[:, n * 512:(n + 1) * 512],
                                lhsT=outTb_sb[:, k * 128:(k + 1) * 128],
                                rhs=dw1_sb[:, (k + 8) * H + n * 512:
                                           (k + 8) * H + (n + 1) * 512],
                                start=(k == 0), stop=False)
                    for n in range(2):
                        nc.tensor.matmul(out=o2_ps[:, n * 512:(n + 1) * 512],
                                         lhsT=ones_b[:],
                                         rhs=db1_sb[:, n * 512:(n + 1) * 512],
                                         start=False, stop=False)
                    ctx_sb = o2w.tile([B, H], bf16)
                    nc.sync.dma_start(out=ctx_sb[:], in_=ctxo[:])
                    ctxT = o2w.tile([128, H], bf16)
                    for k in range(8):
                        trp = ps_tr.tile([128, 128], bf16, tag="tr2")
                        nc.tensor.transpose(out=trp[:],
                                            in_=ctx_sb[:, k * 128:(k + 1) * 128],
                                            identity=idb[:])
                        nc.vector.tensor_copy(out=ctxT[:, k * 128:(k + 1) * 128],
                                              in_=trp[:])
                    for k in range(8):
                        for n in range(2):
                            nc.tensor.matmul(
                                out=o2_ps[:, n * 512:(n + 1) * 512],
                                lhsT=ctxT[:, k * 128:(k + 1) * 128],
                                rhs=dw1_sb[:, k * H + n * 512:k * H + (n + 1) * 512],
                                start=False, stop=(k == 7))
                    nc.scalar.activation(out=o2b[:], in_=o2_ps[:], func=AF.Tanh)
                    for k in range(8):
                        trp = ps_tr.tile([128, 128], bf16, tag="tr2")
                        nc.tensor.transpose(out=trp[:],
                                            in_=o2b[:, k * 128:(k + 1) * 128],
                                            identity=idb[:])
                        nc.vector.tensor_copy(out=o2T[:, k * 128:(k + 1) * 128],
                                              in_=trp[:])

            # ==================== vocab ====================
            # two 2048-col halves; within a half, k outer / n inner so each
            # lhsT streak shares its weight load and 4 PSUM banks accumulate.
            with tc.tile_pool(name="ps_y", bufs=2, space="PSUM") as ps_y, \
                 tc.tile_pool(name="ysb", bufs=2) as ysb:
                for h2 in range(2):
                    hw = 2048 if h2 == 0 else VS - 2048   # 2048 | 1952
                    y_ps = ps_y.tile([128, 2048], f32, tag="y")
                    for k in range(8):
                        for n in range(4):
                            c0c = h2 * 2048 + n * 512
                            cw = min(512, VS - c0c)
                            nc.tensor.matmul(
                                out=y_ps[:, n * 512:n * 512 + cw],
                                lhsT=o2T[:, k * 128:(k + 1) * 128],
                                rhs=dw2_sb[:, k * VS + c0c:k * VS + c0c + cw],
                                start=(k == 0), stop=(k == 7))
                    y_sb = ysb.tile([128, 2048], f32, tag="ysb")
                    nc.vector.tensor_copy(out=y_sb[:, 0:hw], in_=y_ps[:, 0:hw])
                    nc.sync.dma_start(out=yout[:, h2 * 2048:h2 * 2048 + hw],
                                      in_=y_sb[:, 0:hw])

    nc.compile()
    return nc


def _prep_inputs(inputs):
    """Host-side layout: returns list of per-core in_maps."""
    import ml_dtypes
    bf16 = ml_dtypes.bfloat16

    enc = np.asarray(inputs["encoder_output"], np.float32)      # [T, B, H]
    h0 = np.asarray(inputs["h0"], np.float32)
    c0 = np.asarray(inputs["c0"], np.float32)
    emb = np.asarray(inputs["emb"], np.float32)
    Wih = np.asarray(inputs["Wih"], np.float32)
    Whh = np.asarray(inputs["Whh"], np.float32)
    bih = np.asarray(inputs["bih"], np.float32)
    bhh = np.asarray(inputs["bhh"], np.float32)
    aW1 = np.asarray(inputs["aW1"], np.float32)
    aW2 = np.asarray(inputs["aW2"], np.float32)
    ab2 = np.asarray(inputs["ab2"], np.float32)
    dW1 = np.asarray(inputs["dW1"], np.float32)
    db1 = np.asarray(inputs["db1"], np.float32)
    dW2 = np.asarray(inputs["dW2"], np.float32)
    word = np.asarray(inputs["word"]).astype(np.int64)

    x0 = emb[word[0]]                                            # [B, H]
    x0T = np.ascontiguousarray(x0.T).astype(bf16)
    h0T = np.ascontiguousarray(h0.transpose(0, 2, 1)).reshape(NL * H, B).astype(bf16)

    ident_b = np.eye(128, dtype=np.float32).astype(bf16)
    ident_f = np.eye(128, dtype=np.float32)
    iotaL = np.tile(np.arange(L, dtype=np.float32).reshape(1, L), (BS, 1))
    iotaRow = np.zeros((128, NG), np.float32)
    for g in range(NG):
        for p in range(128):
            r = g * 128 + p
            iotaRow[p, g] = float(r if r < ROWS else 0)
    clampR = ((T - 1) * BS + (np.arange(128) % BS)).astype(np.float32).reshape(128, 1)
    selMf = np.zeros((128, BS), np.float32)
    for p in range(128):
        selMf[p, p % BS] = 1.0
    maskC = np.zeros((128, 24), np.float32)
    for p in range(128):
        for c in range(24):
            if p // BS == c % 8:
                maskC[p, c] = 1.0
    lmap = np.zeros((24, NG * 128), np.float32)
    for g in range(NG):
        for row in range(128):
            lmap[g * 8 + row // BS, g * 128 + row] = 1.0

    dW1T = np.ascontiguousarray(dW1.T).astype(bf16)              # [2H, H]
    aW1T = np.ascontiguousarray(aW1.T)                           # [H, HALF] f32
    aW2r = np.tile(aW2.reshape(1, HALF), (128, 1)).astype(np.float32)
    ab2r = np.tile(ab2.reshape(1, 1), (128, 1)).astype(np.float32)
    db1r = db1.reshape(1, H).astype(bf16)
    ab1r = np.asarray(inputs["ab1"], np.float32).reshape(1, HALF).astype(bf16)

    GATE_ORDER = [0, 1, 3, 2]   # [i, f, o, g] so one sigmoid covers i/f/o
    in_maps = []
    for r in range(NC):
        hs = slice(r * HS, (r + 1) * HS)
        rows = np.concatenate([np.arange(g * H + r * HS, g * H + (r + 1) * HS)
                               for g in GATE_ORDER])
        wT_l = []
        gb = np.zeros((NL, 512), np.float32)
        for l in range(NL):
            wT_l.append(np.ascontiguousarray(Wih[l][rows, :].T))  # [H, 512]
            wT_l.append(np.ascontiguousarray(Whh[l][rows, :].T))
            gb[l] = bih[l][rows] + bhh[l][rows]
        wT = np.concatenate(wT_l, axis=0).astype(bf16)           # [NL*2*H, 512]
        c0s = np.ascontiguousarray(
            np.stack([c0[l][:, hs] for l in range(NL)], axis=1).reshape(B, NL * HS))
        bs = slice(r * BS, (r + 1) * BS)
        encs = np.ascontiguousarray(enc[:, bs, :]).reshape(T * BS, H).astype(bf16)
        vs = slice(r * VS, (r + 1) * VS)
        dW2T = np.ascontiguousarray(dW2[vs, :].T).astype(bf16)   # [H, VS]
        repM = np.zeros((128, 128), np.float32)
        for m in range(128):
            repM[r * BS + (m % BS), m] = 1.0
        in_maps.append({
            "x0T": np.ascontiguousarray(x0T),
            "h0T": h0T, "c0s": c0s, "wT": wT,
            "gbias": gb.reshape(1, NL * 512).astype(bf16),
            "aW1T": aW1T, "aW2r": aW2r, "ab2": ab2r,
            "dW1T": dW1T, "db1r": db1r, "ab1r": ab1r, "dW2T": dW2T,
            "encs": encs, "identb": ident_b, "identf": ident_f,
            "iotaL": iotaL, "iotaRow": iotaRow, "clampR": clampR,
            "repMc": repM, "selMfc": selMf,
            "maskCc": maskC, "lmapc": lmap,
        })
    return in_maps


def kernel(**inputs):
    from concourse import bass_utils
    S_val = float(np.asarray(inputs["S"]))
    key = ("mod", S_val)
    if key not in _CACHE:
        _CACHE[key] = _build(S_val)
    nc = _CACHE[key]
    in_maps = _prep_inputs(inputs)
    res = bass_utils.run_bass_kernel_spmd(nc, in_maps, core_ids=list(range(NC)))
    y = np.concatenate([res.results[r]["yout"] for r in range(NC)], axis=1)
    y = y + np.asarray(inputs["db2"], np.float32).reshape(1, V)
    return y.reshape(1, B, V).astype(np.float32)


# revision 8
# speedup vs baseline: 1.2879x; 1.0685x over previous
"""Trainium2 Bass kernel for nn_Decoder_14139032338987 (sparse_attention).

One decoder step: embedding -> 4-layer LSTM -> Gaussian local-window
attention -> output projection -> vocab logits.  B=128, H=1024, V=32000.

Distribution over 8 NeuronCores (B kept whole on every core):
  - LSTM: tensor-parallel.  Core r computes a 128-wide h-slice of all four
    gates, producing x'[:, slice_r]; slices are transposed on-chip and
    AllGathered between layers (the AG output [1024,128] is exactly the
    transposed activation the next layer's matmul needs as lhsT).
  - Attention: p-chain replicated (needs full `out`), window gather and
    softmax sharded over B (16 rows/core) with (l,b)-packed partition
    layout; ctx re-assembled with a 0/1 selector matmul, AllGathered.
  - o2 projection replicated; vocab projection sharded over V (4000/core).
Host does layout only: embedding row gather, weight transposes/casts,
slicing, final concat of the per-core logit slices, and the vocab bias add.

Scheduling notes (from trace analysis):
  - DMA issue order is critical-path order: LSTM inputs first (split per
    layer), aW1 next; dW1 triggers right before the out-AG, dW2 triggers
    after layer-1's load-back so the big loads never starve layer 0.
  - All bias adds are K=1 bf16 matmuls (ones row x bias row) -- fp32
    bias matmuls run in LOW_HIGH mode and cost ~6x more PE time.
  - Gate layout is [i,f,o,g] so one sigmoid covers i/f/o.
  - Per layer the h-half matmuls are issued before the x-half so they
    execute during the previous layer's AllGather gap.
  - The out AllGather carries only the f32 outT chunk (64KB); the
    replicated out-rows needed for scores are rebuilt on-chip via PE
    transposes + a 0/1 selector matmul.
  - Encoder windows are gathered in bf16 (2KB rows) and consumed directly
    by both the score reduction and the ctx matmul.
"""

import numpy as np

H = 1024
V = 32000
NL = 4
W = 10
B = 128
T = 532
L = 2 * W + 1  # 21
HALF = 512
STD2 = (W / 2.0) ** 2  # 25.0
NC = 8
HS = H // NC     # 128 h-slice per core
BS = B // NC     # 16 batch rows per core
VS = V // NC     # 4000 vocab rows per core
NG = (L * BS + 127) // 128  # 3 gather groups of (l,b) rows
ROWS = L * BS  # 336

_CACHE = {}


def _build(S_val: float):
    import concourse.bass as bass
    import concourse.mybir as mybir
    import concourse.bacc as bacc
    import concourse.tile as tile

    dt = mybir.dt
    f32, bf16, i32 = dt.float32, dt.bfloat16, dt.int32
    AF = mybir.ActivationFunctionType
    OP = mybir.AluOpType
    AP = bass.AP
    IOA = bass.IndirectOffsetOnAxis

    nc = bacc.Bacc("TRN2", target_bir_lowering=False, debug=False,
                   enable_asserts=False, num_devices=NC)

    def din(name, shape, d):
        return nc.dram_tensor(name, shape, d, kind="ExternalInput").ap()

    # ---- inputs (per-core data supplied via in_maps) ----
    x0T = din("x0T", [H, B], bf16)
    h0T = din("h0T", [NL * H, B], bf16)
    c0s = din("c0s", [B, NL * HS], f32)
    wT = din("wT", [NL * 2 * H, 512], bf16)        # (l, src, k) tiles
    gbias = din("gbias", [1, NL * 512], bf16)
    aW1T = din("aW1T", [H, HALF], f32)
    aW2r = din("aW2r", [128, HALF], f32)
    ab2 = din("ab2", [128, 1], f32)
    dW1T = din("dW1T", [2 * H, H], bf16)
    db1r = din("db1r", [1, H], bf16)
    ab1r = din("ab1r", [1, HALF], bf16)
    dW2T = din("dW2T", [H, VS], bf16)
    encs = din("encs", [T * BS, H], bf16)
    identb = din("identb", [128, 128], bf16)
    identf = din("identf", [128, 128], f32)
    iotaL = din("iotaL", [BS, L], f32)
    iotaRow = din("iotaRow", [128, NG], f32)
    clampR = din("clampR", [128, 1], f32)
    repMc = din("repMc", [128, 128], f32)
    selMfc = din("selMfc", [128, BS], f32)
    maskCc = din("maskCc", [128, 24], f32)
    lmapc = din("lmapc", [24, NG * 128], f32)

    yout = nc.dram_tensor("yout", [B, VS], f32, kind="ExternalOutput").ap()

    RG = [list(range(NC))]

    with tile.TileContext(nc) as tc:
        with tc.tile_pool(name="const", bufs=1) as cp, \
             tc.tile_pool(name="dw2p", bufs=1) as dw2p, \
             tc.tile_pool(name="dram", bufs=1, space="DRAM") as dp, \
             tc.tile_pool(name="work", bufs=1) as wk:

            # ---- critical-path loads first: LSTM inputs in layer order ----
            ones_b = cp.tile([1, 128], bf16)
            nc.vector.memset(ones_b[:], 1.0)
            idb = cp.tile([128, 128], bf16)
            nc.sync.dma_start(out=idb[:], in_=identb[:])
            idf = cp.tile([128, 128], f32)
            nc.sync.dma_start(out=idf[:], in_=identf[:])
            gb_sb = cp.tile([1, NL * 512], bf16)
            nc.sync.dma_start(out=gb_sb[:], in_=gbias[:])

            # LSTM weight/state tiles (own pool so SBUF frees before o2)
            with tc.tile_pool(name="lstmw", bufs=1) as lw, \
                 tc.tile_pool(name="lstm_work", bufs=1) as lwk, \
                 tc.tile_pool(name="ps_g", bufs=2, space="PSUM") as ps_g:
                xT_sb = lwk.tile([128, H], bf16, tag="xT", bufs=2)
                nc.sync.dma_start(
                    out=xT_sb[:].rearrange("p (k b) -> p k b", b=128),
                    in_=x0T[:].rearrange("(k p) b -> p k b", p=128))
                h0T_sb = lwk.tile([128, NL * 8 * 128], bf16)
                for hh in range(2):
                    nc.sync.dma_start(
                        out=h0T_sb[:, hh * 2048:(hh + 1) * 2048].rearrange(
                            "p (m b) -> p m b", b=128),
                        in_=h0T[hh * 2 * H:(hh + 1) * 2 * H, :].rearrange(
                            "(m p) b -> p m b", p=128))
                w_sb = lw.tile([128, NL * 2 * 8 * 512], bf16)
                c0_sb = lwk.tile([B, NL * HS], f32)
                for l in range(NL):
                    nc.sync.dma_start(
                        out=w_sb[:, l * 8192:(l + 1) * 8192].rearrange(
                            "p (m c) -> p m c", c=512),
                        in_=wT[l * 2 * H:(l + 1) * 2 * H, :].rearrange(
                            "(m p) c -> p m c", p=128))
                    if l == 0:
                        nc.sync.dma_start(out=c0_sb[:, 0:2 * HS],
                                          in_=c0s[:, 0:2 * HS])
                    elif l == 1:
                        nc.sync.dma_start(out=c0_sb[:, 2 * HS:],
                                          in_=c0s[:, 2 * HS:])

                # attention p-chain weights: needed right after the LSTM.
                # hi/lo bf16 split of aW1 (and of out below): q is computed
                # as hi@Wh + hi@Wl + lo@Wh -- ~f32 precision at bf16 speed.
                aw1h_sb = cp.tile([128, 8 * HALF], bf16)
                nc.sync.dma_start(
                    out=aw1h_sb[:].rearrange("p (k c) -> p k c", c=HALF),
                    in_=aW1Th[:].rearrange("(k p) c -> p k c", p=128))
                aw1l_sb = cp.tile([128, 8 * HALF], bf16)
                nc.sync.dma_start(
                    out=aw1l_sb[:].rearrange("p (k c) -> p k c", c=HALF),
                    in_=aW1Tl[:].rearrange("(k p) c -> p k c", p=128))
                # small consts (cheap, after the big critical loads)
                iotaL_sb = cp.tile([BS, L], f32)
                nc.sync.dma_start(out=iotaL_sb[:], in_=iotaL[:])
                ab1_sb = cp.tile([1, HALF], bf16)
                nc.sync.dma_start(out=ab1_sb[:], in_=ab1r[:])
                iotaR_sb = cp.tile([128, NG], f32)
                nc.sync.dma_start(out=iotaR_sb[:], in_=iotaRow[:])
                clamp_sb = cp.tile([128, 1], f32)
                nc.sync.dma_start(out=clamp_sb[:], in_=clampR[:])
                repM_sb = cp.tile([128, 128], f32)
                nc.sync.dma_start(out=repM_sb[:], in_=repMc[:])
                selMf_sb = cp.tile([128, BS], f32)
                nc.sync.dma_start(out=selMf_sb[:], in_=selMfc[:])
                maskC_sb = cp.tile([128, 24], f32)
                nc.sync.dma_start(out=maskC_sb[:], in_=maskCc[:])
                lmap_sb = cp.tile([24, NG * 128], f32)
                nc.sync.dma_start(out=lmap_sb[:], in_=lmapc[:])
                aw2_sb = cp.tile([128, HALF], f32)
                nc.sync.dma_start(out=aw2_sb[:], in_=aW2r[:])
                ab2_sb = cp.tile([128, 1], f32)
                nc.sync.dma_start(out=ab2_sb[:], in_=ab2[:])
                db1_sb = cp.tile([1, H], bf16)
                nc.sync.dma_start(out=db1_sb[:], in_=db1r[:])

                # big deferred loads: dW2 streams during layers 2-3 +
                # attention (trigger placed mid-LSTM below)
                dw2_sb = dw2p.tile([128, 8 * VS], bf16)

                outT_sb = wk.tile([128, H], f32)      # final hidden, transposed
                outTb_sb = wk.tile([128, H], bf16)
                o2T = wk.tile([128, H], bf16)
                ago3 = dp.tile([H, 128], f32, name="ago3", addr_space="Shared")

                # ============================ LSTM ============================
                for l in range(NL):
                    g_ps = ps_g.tile([128, 512], f32, tag="g")
                    # h-half first: it has no dependence on the previous
                    # layer's AllGather, so it fills the AG gap on the PE.
                    for k in range(8):
                        nc.tensor.matmul(
                            out=g_ps[:],
                            lhsT=h0T_sb[:, (l * 8 + k) * 128:(l * 8 + k + 1) * 128],
                            rhs=w_sb[:, ((l * 2 + 1) * 8 + k) * 512:
                                     ((l * 2 + 1) * 8 + k + 1) * 512],
                            start=(k == 0), stop=False)
                    nc.tensor.matmul(out=g_ps[:], lhsT=ones_b[:],
                                     rhs=gb_sb[:, l * 512:(l + 1) * 512],
                                     start=False, stop=False)
                    for k in range(8):
                        nc.tensor.matmul(
                            out=g_ps[:],
                            lhsT=xT_sb[:, k * 128:(k + 1) * 128],
                            rhs=w_sb[:, ((l * 2) * 8 + k) * 512:
                                     ((l * 2) * 8 + k + 1) * 512],
                            start=False, stop=(k == 7))
                    # gates packed [i,f,o,g]: one sigmoid covers i/f/o
                    ifo = lwk.tile([128, 384], f32, tag="ifo")
                    nc.scalar.activation(out=ifo[:], in_=g_ps[:, 0:384],
                                         func=AF.Sigmoid)
                    g_t = lwk.tile([128, 128], f32, tag="g_t")
                    nc.scalar.activation(out=g_t[:], in_=g_ps[:, 384:512],
                                         func=AF.Tanh)
                    cnew = lwk.tile([128, 128], f32, tag="cnew")
                    nc.vector.tensor_mul(out=cnew[:], in0=ifo[:, 128:256],
                                         in1=c0_sb[:, l * HS:(l + 1) * HS])
                    ig = lwk.tile([128, 128], f32, tag="ig")
                    nc.vector.tensor_mul(out=ig[:], in0=ifo[:, 0:128], in1=g_t[:])
                    nc.vector.tensor_add(out=cnew[:], in0=cnew[:], in1=ig[:])
                    tc_t = lwk.tile([128, 128], f32, tag="tc_t")
                    nc.scalar.activation(out=tc_t[:], in_=cnew[:], func=AF.Tanh)
                    xf = lwk.tile([128, 128], f32, tag="xf")
                    nc.vector.tensor_mul(out=xf[:], in0=ifo[:, 256:384],
                                         in1=tc_t[:])

                    if l < NL - 1:
                        xb = lwk.tile([128, 128], bf16, tag="xb")
                        nc.vector.tensor_copy(out=xb[:], in_=xf[:])
                        tr_ps = ps_g.tile([128, 128], bf16, tag="tr")
                        nc.tensor.transpose(out=tr_ps[:], in_=xb[:], identity=idb[:])
                        xTs = lwk.tile([128, 128], bf16, tag="xTs")
                        nc.vector.tensor_copy(out=xTs[:], in_=tr_ps[:])
                        agi = dp.tile([128, 128], bf16, name=f"agi{l}", tag=f"agi{l}")
                        nc.sync.dma_start(out=agi[:], in_=xTs[:])
                        ago = dp.tile([H, 128], bf16, name=f"ago{l}", tag=f"ago{l}",
                                      addr_space="Shared")
                        nc.gpsimd.collective_compute(
                            "AllGather", OP.bypass, replica_groups=RG,
                            ins=[agi[:]], outs=[ago[:]])
                        xT_sb = lwk.tile([128, H], bf16, tag="xT", bufs=2)
                        nc.sync.dma_start(
                            out=xT_sb[:].rearrange("p (k b) -> p k b", b=128),
                            in_=ago[:].rearrange("(k p) b -> p k b", p=128))
                        if l == 1:
                            # dW2 (8MB) trigger sits after layer-1's
                            # load-back: transfer runs during layers 2-3 +
                            # attention, never contending with LSTM loads.
                            nc.sync.dma_start(
                                out=dw2_sb[:].rearrange("p (k c) -> p k c", c=VS),
                                in_=dW2T[:].rearrange("(k p) c -> p k c", p=128))
                    else:
                        tr_ps = ps_g.tile([128, 128], f32, tag="trf")
                        nc.tensor.transpose(out=tr_ps[:], in_=xf[:], identity=idf[:])
                        agi3 = dp.tile([128, 128], f32, name="agi3")
                        xTf = lwk.tile([128, 128], f32, tag="xTf")
                        nc.vector.tensor_copy(out=xTf[:], in_=tr_ps[:])
                        nc.sync.dma_start(out=agi3[:], in_=xTf[:])
                        nc.gpsimd.collective_compute(
                            "AllGather", OP.bypass, replica_groups=RG,
                            ins=[agi3[:]], outs=[ago3[:]])

            # ---- dW1: trigger before the out-AG load-back; the 4MB
            # transfer overlaps the AllGather itself.
            with tc.tile_pool(name="dw1p", bufs=1) as dw1p:
                dw1_sb = dw1p.tile([128, 16 * H], bf16)
                nc.sync.dma_start(
                    out=dw1_sb[:].rearrange("p (m c) -> p m c", c=H),
                    in_=dW1T[:].rearrange("(m p) c -> p m c", p=128))
                nc.sync.dma_start(
                    out=outT_sb[:].rearrange("p (k b) -> p k b", b=128),
                    in_=ago3[:].rearrange("(k p) b -> p k b", p=128))
                nc.vector.tensor_copy(out=outTb_sb[:], in_=outT_sb[:])

                # ============================ p-chain ============================
                with tc.tile_pool(name="att", bufs=1) as at, \
                     tc.tile_pool(name="ps_a", bufs=1, space="PSUM") as ps_a:
                    pt_ps = ps_a.tile([128, HALF], f32, tag="pt")
                    for k in range(8):
                        nc.tensor.matmul(out=pt_ps[:],
                                         lhsT=outT_sb[:, k * 128:(k + 1) * 128],
                                         rhs=aw1_sb[:, k * HALF:(k + 1) * HALF],
                                         start=(k == 0), stop=False)
                    nc.tensor.matmul(out=pt_ps[:], lhsT=ones_b[:], rhs=ab1_sb[:],
                                     start=False, stop=True)
                    pt = at.tile([128, HALF], f32)
                    nc.scalar.activation(out=pt[:], in_=pt_ps[:], func=AF.Tanh)
                    scr5 = at.tile([128, HALF], f32)
                    z = at.tile([128, 1], f32)
                    nc.vector.scalar_tensor_tensor(
                        out=scr5[:], in0=pt[:], scalar=1.0,
                        in1=aw2_sb[:], op0=OP.mult, op1=OP.mult,
                        accum_out=z[:])
                    sg = at.tile([128, 1], f32)
                    nc.scalar.activation(out=sg[:], in_=z[:], func=AF.Sigmoid,
                                         bias=ab2_sb[:])
                    s_f = at.tile([128, 1], f32)       # p - W  (pre-round)
                    nc.vector.tensor_scalar_mul(out=s_f[:], in0=sg[:],
                                                scalar1=float(S_val))
                    r_ = at.tile([128, 1], f32)
                    nc.vector.tensor_scalar_add(out=r_[:], in0=s_f[:], scalar1=0.5)
                    # floor(r_) robust to the f32->i32 cast rounding mode:
                    # f = cast(r_); if f > r_ then f -= 1
                    ti = at.tile([128, 1], i32)
                    nc.vector.tensor_copy(out=ti[:], in_=r_[:])
                    tf = at.tile([128, 1], f32)
                    nc.vector.tensor_copy(out=tf[:], in_=ti[:])
                    cond = at.tile([128, 1], f32)
                    nc.vector.tensor_tensor(out=cond[:], in0=tf[:], in1=r_[:],
                                            op=OP.is_gt)
                    stf = at.tile([128, 1], f32)       # start (rounded, float)
                    nc.vector.tensor_sub(out=stf[:], in0=tf[:], in1=cond[:])
                    d0 = at.tile([128, 1], f32)        # start - p  (= stf - s_f - W)
                    nc.vector.tensor_sub(out=d0[:], in0=stf[:], in1=s_f[:])
                    nc.vector.tensor_scalar_add(out=d0[:], in0=d0[:],
                                                scalar1=-float(W))

                    # ---- on-chip replicated out rows (this core's 16 b) ----
                    # rebuild out[b,h] via 8 full PE transposes of outT
                    # chunks, then orep = repM.T @ out replicates this
                    # core's 16 rows to the 128 (l,b) partition rows
                    # (repM is a per-core host constant, program is SPMD).
                    out_full = at.tile([128, H], f32)
                    for k in range(8):
                        tf_ps = ps_a.tile([128, 128], f32, tag="t16", bufs=2)
                        nc.tensor.transpose(
                            out=tf_ps[:],
                            in_=outT_sb[:, k * 128:(k + 1) * 128],
                            identity=idf[:])
                        nc.vector.tensor_copy(out=out_full[:, k * 128:(k + 1) * 128],
                                              in_=tf_ps[:])
                    orep_ps = ps_a.tile([128, H], f32, tag="orep")
                    for n in range(2):
                        nc.tensor.matmul(out=orep_ps[:, n * 512:(n + 1) * 512],
                                         lhsT=repM_sb[:],
                                         rhs=out_full[:, n * 512:(n + 1) * 512],
                                         start=True, stop=True)
                    orep = at.tile([128, H], f32)
                    nc.vector.tensor_copy(out=orep[:, 0:512], in_=orep_ps[:, 0:512])
                    nc.vector.tensor_copy(out=orep[:, 512:1024],
                                          in_=orep_ps[:, 512:1024])

                    # ---- cross-partition replication via 0/1 matmuls ----
                    pk2 = at.tile([128, 2], f32)
                    nc.vector.tensor_copy(out=pk2[:, 0:1], in_=stf[:])
                    nc.vector.tensor_copy(out=pk2[:, 1:2], in_=d0[:])
                    g16_ps = ps_a.tile([BS, 2], f32, tag="scr", bufs=1)
                    nc.tensor.matmul(out=g16_ps[:], lhsT=repM_sb[:, 0:BS],
                                     rhs=pk2[:], start=True, stop=True)
                    g16 = at.tile([BS, 2], f32)
                    nc.vector.tensor_copy(out=g16[:], in_=g16_ps[:])
                    stf16 = g16[:, 0:1]
                    d016 = g16[:, 1:2]
                    str_ps = ps_a.tile([128, 1], f32, tag="scr", bufs=1)
                    nc.tensor.matmul(out=str_ps[:], lhsT=repM_sb[:],
                                     rhs=stf[:], start=True, stop=True)
                    stf_rep = at.tile([128, 1], f32)
                    nc.vector.tensor_copy(out=stf_rep[:], in_=str_ps[:])

                    # ==================== gather ====================
                    sel = [at.tile([128, H], bf16, name=f"sel{g}", tag=f"sel{g}")
                           for g in range(NG)]
                    cnt = [128, 128, ROWS - 256]
                    idxs = []
                    for g in range(NG):
                        idxf = at.tile([128, 1], f32, tag=f"idxf{g}")
                        nc.vector.tensor_scalar_mul(out=idxf[:], in0=stf_rep[:],
                                                    scalar1=float(BS))
                        nc.vector.tensor_add(out=idxf[:], in0=idxf[:],
                                             in1=iotaR_sb[:, g:g + 1])
                        nc.vector.tensor_tensor(out=idxf[:], in0=idxf[:],
                                                in1=clamp_sb[:], op=OP.min)
                        idx = at.tile([128, 1], i32, tag=f"idx{g}")
                        nc.vector.tensor_copy(out=idx[:], in_=idxf[:])
                        idxs.append(idx)
                    for g in range(NG):
                        nc.gpsimd.indirect_dma_start(
                            out=sel[g][0:cnt[g], :], out_offset=None,
                            in_=encs[:],
                            in_offset=IOA(ap=idxs[g][0:cnt[g], :1], axis=0))

                    # ---- window masks + gaussian (overlap the gathers) ----
                    pos = at.tile([BS, L], f32)
                    nc.vector.tensor_scalar(out=pos[:], in0=iotaL_sb[:],
                                            scalar1=stf16, scalar2=None, op0=OP.add)
                    v1 = at.tile([BS, L], f32)
                    nc.vector.tensor_scalar(out=v1[:], in0=pos[:], scalar1=float(W),
                                            scalar2=None, op0=OP.is_ge)
                    v2 = at.tile([BS, L], f32)
                    nc.vector.tensor_scalar(out=v2[:], in0=pos[:],
                                            scalar1=float(S_val + W),
                                            scalar2=None, op0=OP.is_lt)
                    nc.vector.tensor_mul(out=v1[:], in0=v1[:], in1=v2[:])
                    dd = at.tile([BS, L], f32)
                    nc.vector.tensor_scalar(out=dd[:], in0=iotaL_sb[:],
                                            scalar1=d016, scalar2=None, op0=OP.add)
                    d2 = at.tile([BS, L], f32)
                    nc.vector.tensor_mul(out=d2[:], in0=dd[:], in1=dd[:])
                    gs = at.tile([BS, L], f32)
                    nc.scalar.activation(out=gs[:], in_=d2[:], func=AF.Exp,
                                         scale=-1.0 / (2.0 * STD2))

                    # ==================== scores ====================
                    sc_col = at.tile([128, NG], f32)
                    nc.vector.memset(sc_col[:], 0.0)
                    scrH = at.tile([128, H], f32)
                    for g in range(NG):
                        nc.vector.scalar_tensor_tensor(
                            out=scrH[0:cnt[g], :], in0=orep[0:cnt[g], :], scalar=1.0,
                            in1=sel[g][0:cnt[g], :], op0=OP.mult, op1=OP.mult,
                            accum_out=sc_col[0:cnt[g], g:g + 1])

                    # -------- [16, 21] softmax block --------
                    X = at.tile([128, 24], f32)
                    nc.vector.tensor_tensor(
                        out=X[:].rearrange("p (g li) -> p g li", g=NG),
                        in0=sc_col[:].unsqueeze(2).broadcast_to([128, NG, 8]),
                        in1=maskC_sb[:].rearrange("p (g li) -> p g li", g=NG),
                        op=OP.mult)
                    sc_ps = ps_a.tile([BS, 24], f32, tag="scr", bufs=1)
                    nc.tensor.matmul(out=sc_ps[:], lhsT=selMf_sb[:], rhs=X[:],
                                     start=True, stop=True)
                    sc24 = at.tile([BS, 24], f32)
                    nc.vector.tensor_copy(out=sc24[:], in_=sc_ps[:])
                    sc16 = sc24[:, 0:L]

                    sm = at.tile([BS, L], f32)
                    nc.vector.tensor_scalar_add(out=sm[:], in0=sc16, scalar1=-1e-12)
                    nc.vector.tensor_mul(out=sm[:], in0=sm[:], in1=v1[:])
                    nc.vector.tensor_scalar_add(out=sm[:], in0=sm[:], scalar1=1e-12)
                    mx = at.tile([BS, 1], f32)
                    nc.vector.tensor_reduce(out=mx[:], in_=sm[:],
                                            axis=mybir.AxisListType.X, op=OP.max)
                    nmx = at.tile([BS, 1], f32)
                    nc.vector.tensor_scalar_mul(out=nmx[:], in0=mx[:], scalar1=-1.0)
                    ex = at.tile([BS, L], f32)
                    se = at.tile([BS, 1], f32)
                    nc.scalar.activation(out=ex[:], in_=sm[:], func=AF.Exp,
                                         bias=nmx[:], accum_out=se[:])
                    ri = at.tile([BS, 1], f32)
                    nc.vector.reciprocal(out=ri[:], in_=se[:])
                    aa = at.tile([BS, L], f32)
                    nc.vector.tensor_scalar(out=aa[:], in0=ex[:], scalar1=ri[:],
                                            scalar2=None, op0=OP.mult)
                    nc.vector.tensor_mul(out=aa[:], in0=aa[:], in1=gs[:])
                    # relayout a -> a-weighted selector Sa via PE
                    aa24 = at.tile([BS, 24], f32)
                    nc.vector.memset(aa24[:], 0.0)
                    nc.vector.tensor_copy(out=aa24[:, 0:L], in_=aa[:])
                    aaT_ps = ps_a.tile([24, BS], f32, tag="scr", bufs=1)
                    nc.tensor.transpose(out=aaT_ps[:], in_=aa24[:],
                                        identity=idf[0:BS, 0:BS])
                    aaT = at.tile([24, BS], f32)
                    nc.vector.tensor_copy(out=aaT[:], in_=aaT_ps[:])

                    # ==================== ctx ====================
                    ctx_ps = ps_a.tile([BS, H], f32, tag="ctx")
                    for g in range(NG):
                        sa_ps = ps_a.tile([128, BS], f32, tag="scr", bufs=1)
                        nc.tensor.matmul(out=sa_ps[:],
                                         lhsT=lmap_sb[:, g * 128:(g + 1) * 128],
                                         rhs=aaT[:], start=True, stop=True)
                        sa = at.tile([128, BS], bf16, tag="sab", bufs=3)
                        nc.vector.tensor_mul(out=sa[:], in0=sa_ps[:],
                                             in1=selMf_sb[:])
                        for n in range(2):
                            nc.tensor.matmul(
                                out=ctx_ps[:, n * 512:(n + 1) * 512],
                                lhsT=sa[0:cnt[g], :],
                                rhs=sel[g][0:cnt[g], n * 512:(n + 1) * 512],
                                start=(g == 0), stop=(g == NG - 1))
                    ctxb = at.tile([BS, H], bf16)
                    nc.vector.tensor_copy(out=ctxb[:], in_=ctx_ps[:])
                    ctxi = dp.tile([BS, H], bf16, name="ctxi")
                    nc.sync.dma_start(out=ctxi[:], in_=ctxb[:])
                ctxo = dp.tile([B, H], bf16, name="ctxo", addr_space="Shared")
                nc.gpsimd.collective_compute(
                    "AllGather", OP.bypass, replica_groups=RG,
                    ins=[ctxi[:]], outs=[ctxo[:]])

                # ==================== o2 ====================
                o2b = wk.tile([128, H], bf16)
                with tc.tile_pool(name="o2w", bufs=1) as o2w, \
                     tc.tile_pool(name="ps_o2", bufs=1, space="PSUM") as ps_o2, \
                     tc.tile_pool(name="ps_tr", bufs=2, space="PSUM") as ps_tr:
                    o2_ps = ps_o2.tile([128, H], f32, tag="o2")
                    # out-half first: runs during the ctx AllGather
                    for k in range(8):
                        for n in range(2):
                            nc.tensor.matmul(
                                out=o2_ps[:, n * 512:(n + 1) * 512],
                                lhsT=outTb_sb[:, k * 128:(k + 1) * 128],
                                rhs=dw1_sb[:, (k + 8) * H + n * 512:
                                           (k + 8) * H + (n + 1) * 512],
                                start=(k == 0), stop=False)
                    for n in range(2):
                        nc.tensor.matmul(out=o2_ps[:, n * 512:(n + 1) * 512],
                                         lhsT=ones_b[:],
                                         rhs=db1_sb[:, n * 512:(n + 1) * 512],
                                         start=False, stop=False)
                    ctx_sb = o2w.tile([B, H], bf16)
                    nc.sync.dma_start(out=ctx_sb[:], in_=ctxo[:])
                    ctxT = o2w.tile([128, H], bf16)
                    for k in range(8):
                        trp = ps_tr.tile([128, 128], bf16, tag="tr2")
                        nc.tensor.transpose(out=trp[:],
                                            in_=ctx_sb[:, k * 128:(k + 1) * 128],
                                            identity=idb[:])
                        nc.vector.tensor_copy(out=ctxT[:, k * 128:(k + 1) * 128],
                                              in_=trp[:])
                    for k in range(8):
                        for n in range(2):
                            nc.tensor.matmul(
                                out=o2_ps[:, n * 512:(n + 1) * 512],
                                lhsT=ctxT[:, k * 128:(k + 1) * 128],
                                rhs=dw1_sb[:, k * H + n * 512:k * H + (n + 1) * 512],
                                start=False, stop=(k == 7))
                    nc.scalar.activation(out=o2b[:], in_=o2_ps[:], func=AF.Tanh)
                    for k in range(8):
                        trp = ps_tr.tile([128, 128], bf16, tag="tr2")
                        nc.tensor.transpose(out=trp[:],
                                            in_=o2b[:, k * 128:(k + 1) * 128],
                                            identity=idb[:])
                        nc.vector.tensor_copy(out=o2T[:, k * 128:(k + 1) * 128],
                                              in_=trp[:])

            # ==================== vocab ====================
            # two 2048-col halves; within a half, k outer / n inner so each
            # lhsT streak shares its weight load and 4 PSUM banks accumulate.
            with tc.tile_pool(name="ps_y", bufs=2, space="PSUM") as ps_y, \
                 tc.tile_pool(name="ysb", bufs=2) as ysb:
                for h2 in range(2):
                    hw = 2048 if h2 == 0 else VS - 2048   # 2048 | 1952
                    y_ps = ps_y.tile([128, 2048], f32, tag="y")
                    for k in range(8):
                        for n in range(4):
                            c0c = h2 * 2048 + n * 512
                            cw = min(512, VS - c0c)
                            nc.tensor.matmul(
                                out=y_ps[:, n * 512:n * 512 + cw],
                                lhsT=o2T[:, k * 128:(k + 1) * 128],
                                rhs=dw2_sb[:, k * VS + c0c:k * VS + c0c + cw],
                                start=(k == 0), stop=(k == 7))
                    y_sb = ysb.tile([128, 2048], f32, tag="ysb")
                    nc.vector.tensor_copy(out=y_sb[:, 0:hw], in_=y_ps[:, 0:hw])
                    nc.sync.dma_start(out=yout[:, h2 * 2048:h2 * 2048 + hw],
                                      in_=y_sb[:, 0:hw])

    nc.compile()
    return nc


def _prep_inputs(inputs):
    """Host-side layout: returns list of per-core in_maps."""
    import ml_dtypes
    bf16 = ml_dtypes.bfloat16

    enc = np.asarray(inputs["encoder_output"], np.float32)      # [T, B, H]
    h0 = np.asarray(inputs["h0"], np.float32)
    c0 = np.asarray(inputs["c0"], np.float32)
    emb = np.asarray(inputs["emb"], np.float32)
    Wih = np.asarray(inputs["Wih"], np.float32)
    Whh = np.asarray(inputs["Whh"], np.float32)
    bih = np.asarray(inputs["bih"], np.float32)
    bhh = np.asarray(inputs["bhh"], np.float32)
    aW1 = np.asarray(inputs["aW1"], np.float32)
    aW2 = np.asarray(inputs["aW2"], np.float32)
    ab2 = np.asarray(inputs["ab2"], np.float32)
    dW1 = np.asarray(inputs["dW1"], np.float32)
    db1 = np.asarray(inputs["db1"], np.float32)
    dW2 = np.asarray(inputs["dW2"], np.float32)
    word = np.asarray(inputs["word"]).astype(np.int64)

    x0 = emb[word[0]]                                            # [B, H]
    x0T = np.ascontiguousarray(x0.T).astype(bf16)
    h0T = np.ascontiguousarray(h0.transpose(0, 2, 1)).reshape(NL * H, B).astype(bf16)

    ident_b = np.eye(128, dtype=np.float32).astype(bf16)
    ident_f = np.eye(128, dtype=np.float32)
    iotaL = np.tile(np.arange(L, dtype=np.float32).reshape(1, L), (BS, 1))
    iotaRow = np.zeros((128, NG), np.float32)
    for g in range(NG):
        for p in range(128):
            r = g * 128 + p
            iotaRow[p, g] = float(r if r < ROWS else 0)
    clampR = ((T - 1) * BS + (np.arange(128) % BS)).astype(np.float32).reshape(128, 1)
    selMf = np.zeros((128, BS), np.float32)
    for p in range(128):
        selMf[p, p % BS] = 1.0
    maskC = np.zeros((128, 24), np.float32)
    for p in range(128):
        for c in range(24):
            if p // BS == c % 8:
                maskC[p, c] = 1.0
    lmap = np.zeros((24, NG * 128), np.float32)
    for g in range(NG):
        for row in range(128):
            lmap[g * 8 + row // BS, g * 128 + row] = 1.0

    dW1T = np.ascontiguousarray(dW1.T).astype(bf16)              # [2H, H]
    aW1T = np.ascontiguousarray(aW1.T)                           # [H, HALF] f32
    aW2r = np.tile(aW2.reshape(1, HALF), (128, 1)).astype(np.float32)
    ab2r = np.tile(ab2.reshape(1, 1), (128, 1)).astype(np.float32)
    db1r = db1.reshape(1, H).astype(bf16)
    ab1r = np.asarray(inputs["ab1"], np.float32).reshape(1, HALF).astype(bf16)

    GATE_ORDER = [0, 1, 3, 2]   # [i, f, o, g] so one sigmoid covers i/f/o
    in_maps = []
    for r in range(NC):
        hs = slice(r * HS, (r + 1) * HS)
        rows = np.concatenate([np.arange(g * H + r * HS, g * H + (r + 1) * HS)
                               for g in GATE_ORDER])
        wT_l = []
        gb = np.zeros((NL, 512), np.float32)
        for l in range(NL):
            wT_l.append(np.ascontiguousarray(Wih[l][rows, :].T))  # [H, 512]
            wT_l.append(np.ascontiguousarray(Whh[l][rows, :].T))
            gb[l] = bih[l][rows] + bhh[l][rows]
        wT = np.concatenate(wT_l, axis=0).astype(bf16)           # [NL*2*H, 512]
        c0s = np.ascontiguousarray(
            np.stack([c0[l][:, hs] for l in range(NL)], axis=1).reshape(B, NL * HS))
        bs = slice(r * BS, (r + 1) * BS)
        encs = np.ascontiguousarray(enc[:, bs, :]).reshape(T * BS, H).astype(bf16)
        vs = slice(r * VS, (r + 1) * VS)
        dW2T = np.ascontiguousarray(dW2[vs, :].T).astype(bf16)   # [H, VS]
        repM = np.zeros((128, 128), np.float32)
        for m in range(128):
            repM[r * BS + (m % BS), m] = 1.0
        in_maps.append({
            "x0T": np.ascontiguousarray(x0T),
            "h0T": h0T, "c0s": c0s, "wT": wT,
            "gbias": gb.reshape(1, NL * 512).astype(bf16),
            "aW1T": aW1T, "aW2r": aW2r, "ab2": ab2r,
            "dW1T": dW1T, "db1r": db1r, "ab1r": ab1r, "dW2T": dW2T,
            "encs": encs, "identb": ident_b, "identf": ident_f,
            "iotaL": iotaL, "iotaRow": iotaRow, "clampR": clampR,
            "repMc": repM, "selMfc": selMf,
            "maskCc": maskC, "lmapc": lmap,
        })
    return in_maps


def kernel(**inputs):
    from concourse import bass_utils
    S_val = float(np.asarray(inputs["S"]))
    key = ("mod", S_val)
    if key not in _CACHE:
        _CACHE[key] = _build(S_val)
    nc = _CACHE[key]
    in_maps = _prep_inputs(inputs)
    res = bass_utils.run_bass_kernel_spmd(nc, in_maps, core_ids=list(range(NC)))
    y = np.concatenate([res.results[r]["yout"] for r in range(NC)], axis=1)
    y = y + np.asarray(inputs["db2"], np.float32).reshape(1, V)
    return y.reshape(1, B, V).astype(np.float32)


# revision 11
# speedup vs baseline: 1.3175x; 1.0230x over previous
"""Trainium2 Bass kernel for nn_Decoder_14139032338987 (sparse_attention).

One decoder step: embedding -> 4-layer LSTM -> Gaussian local-window
attention -> output projection -> vocab logits.  B=128, H=1024, V=32000.

Distribution over 8 NeuronCores (B kept whole on every core):
  - LSTM: tensor-parallel.  Core r computes a 128-wide h-slice of all four
    gates, producing x'[:, slice_r]; slices are transposed on-chip and
    AllGathered between layers (the AG output [1024,128] is exactly the
    transposed activation the next layer's matmul needs as lhsT).
  - Attention: p-chain replicated (needs full `out`), window gather and
    softmax sharded over B (16 rows/core) with (l,b)-packed partition
    layout; ctx re-assembled with a 0/1 selector matmul, AllGathered.
  - o2 projection replicated; vocab projection sharded over V (4000/core).
Host does layout only: embedding row gather, weight transposes/casts,
slicing, final concat of the per-core logit slices, and the vocab bias add.

Scheduling notes (from trace analysis):
  - DMA issue order is critical-path order: LSTM inputs first (split per
    layer), aW1 next; dW1 triggers right before the out-AG, dW2 triggers
    after layer-1's load-back so the big loads never starve layer 0.
  - All bias adds are K=1 bf16 matmuls (ones row x bias row) -- fp32
    bias matmuls run in LOW_HIGH mode and cost ~6x more PE time.
  - Gate layout is [i,f,o,g] so one sigmoid covers i/f/o.
  - Per layer the h-half matmuls are issued before the x-half so they
    execute during the previous layer's AllGather gap.
  - The out AllGather carries only the f32 outT chunk (64KB); the
    replicated out-rows needed for scores are rebuilt on-chip via PE
    transposes + a 0/1 selector matmul.
  - Encoder windows are gathered in bf16 (2KB rows) and consumed directly
    by both the score reduction and the ctx matmul.
"""

import numpy as np

H = 1024
V = 32000
NL = 4
W = 10
B = 128
T = 532
L = 2 * W + 1  # 21
HALF = 512
STD2 = (W / 2.0) ** 2  # 25.0
NC = 8
HS = H // NC     # 128 h-slice per core
BS = B // NC     # 16 batch rows per core
VS = V // NC     # 4000 vocab rows per core
NG = (L * BS + 127) // 128  # 3 gather groups of (l,b) rows
ROWS = L * BS  # 336

_CACHE = {}


def _build(S_val: float):
    import concourse.bass as bass
    import concourse.mybir as mybir
    import concourse.bacc as bacc
    import concourse.tile as tile

    dt = mybir.dt
    f32, bf16, i32 = dt.float32, dt.bfloat16, dt.int32
    AF = mybir.ActivationFunctionType
    OP = mybir.AluOpType
    AP = bass.AP
    IOA = bass.IndirectOffsetOnAxis

    nc = bacc.Bacc("TRN2", target_bir_lowering=False, debug=False,
                   enable_asserts=False, num_devices=NC)

    def din(name, shape, d):
        return nc.dram_tensor(name, shape, d, kind="ExternalInput").ap()

    # ---- inputs (per-core data supplied via in_maps) ----
    x0T = din("x0T", [128, 8 * B], bf16)
    h0T = din("h0T", [128, NL * 8 * 128], bf16)
    c0s = din("c0s", [B, NL * HS], f32)
    wT = din("wT", [128, NL * 2 * 8 * 512], bf16)  # (l, src, k) tiles
    gbias = din("gbias", [1, NL * 512], bf16)
    aW1Th = din("aW1Th", [128, 8 * HALF], bf16)
    aW1Tl = din("aW1Tl", [128, 8 * HALF], bf16)
    aW2r = din("aW2r", [128, HALF], f32)
    ab2 = din("ab2", [128, 1], f32)
    dW1T = din("dW1T", [128, 16 * H], bf16)
    db1r = din("db1r", [1, H], bf16)
    ab1r = din("ab1r", [1, HALF], bf16)
    dW2T = din("dW2T", [128, 8 * VS], bf16)
    encs = din("encs", [T * BS, H], bf16)
    identb = din("identb", [128, 128], bf16)
    identf = din("identf", [128, 128], f32)
    iotaL = din("iotaL", [BS, L], f32)
    iotaRow = din("iotaRow", [128, NG], f32)
    clampR = din("clampR", [128, 1], f32)
    repMc = din("repMc", [128, 128], f32)
    selMfc = din("selMfc", [128, BS], f32)
    maskCc = din("maskCc", [128, 24], f32)
    lmapc = din("lmapc", [24, NG * 128], f32)

    yout = nc.dram_tensor("yout", [B, VS], f32, kind="ExternalOutput").ap()

    RG = [list(range(NC))]

    with tile.TileContext(nc) as tc:
        with tc.tile_pool(name="const", bufs=1) as cp, \
             tc.tile_pool(name="dw2p", bufs=1) as dw2p, \
             tc.tile_pool(name="dram", bufs=1, space="DRAM") as dp, \
             tc.tile_pool(name="work", bufs=1) as wk:

            # ---- dummy warm-up AllGather: absorbs the runtime start
            # barrier (launch skew) + first-collective ring setup against
            # the weight-load dead time, so the real AG0 runs warm.
            ones_b = cp.tile([1, 128], bf16)
            nc.vector.memset(ones_b[:], 1.0)
            dumi = dp.tile([1, 16], bf16, name="dumi")
            nc.sync.dma_start(out=dumi[:], in_=ones_b[0:1, 0:16])
            dumo = dp.tile([NC, 16], bf16, name="dumo", addr_space="Shared")
            nc.gpsimd.collective_compute(
                "AllGather", OP.bypass, replica_groups=RG,
                ins=[dumi[:]], outs=[dumo[:]])
            idb = cp.tile([128, 128], bf16)
            nc.sync.dma_start(out=idb[:], in_=identb[:])
            idf = cp.tile([128, 128], f32)
            nc.sync.dma_start(out=idf[:], in_=identf[:])
            gb_sb = cp.tile([1, NL * 512], bf16)
            nc.sync.dma_start(out=gb_sb[:], in_=gbias[:])

            # LSTM weight/state tiles (own pool so SBUF frees before o2)
            with tc.tile_pool(name="lstmw", bufs=1) as lw, \
                 tc.tile_pool(name="lstm_work", bufs=1) as lwk, \
                 tc.tile_pool(name="ps_g", bufs=2, space="PSUM") as ps_g:
                xT_sb = lwk.tile([128, H], bf16, tag="xT", bufs=2)
                nc.sync.dma_start(out=xT_sb[:], in_=x0T[:])
                h0T_sb = lwk.tile([128, NL * 8 * 128], bf16)
                for hh in range(2):
                    nc.sync.dma_start(
                        out=h0T_sb[:, hh * 2048:(hh + 1) * 2048],
                        in_=h0T[:, hh * 2048:(hh + 1) * 2048])
                w_sb = lw.tile([128, NL * 2 * 8 * 512], bf16)
                c0_sb = lwk.tile([B, NL * HS], f32)
                for l in range(NL):
                    nc.sync.dma_start(
                        out=w_sb[:, l * 8192:(l + 1) * 8192],
                        in_=wT[:, l * 8192:(l + 1) * 8192])
                    if l == 0:
                        nc.sync.dma_start(out=c0_sb[:, 0:2 * HS],
                                          in_=c0s[:, 0:2 * HS])
                    elif l == 1:
                        nc.sync.dma_start(out=c0_sb[:, 2 * HS:],
                                          in_=c0s[:, 2 * HS:])

                # attention p-chain weights: needed right after the LSTM.
                # hi/lo bf16 split of aW1 (and of out below): q is computed
                # as hi@Wh + hi@Wl + lo@Wh -- ~f32 precision at bf16 speed.
                aw1h_sb = cp.tile([128, 8 * HALF], bf16)
                nc.sync.dma_start(out=aw1h_sb[:], in_=aW1Th[:])
                aw1l_sb = cp.tile([128, 8 * HALF], bf16)
                nc.sync.dma_start(out=aw1l_sb[:], in_=aW1Tl[:])
                # small consts (cheap, after the big critical loads)
                iotaL_sb = cp.tile([BS, L], f32)
                nc.sync.dma_start(out=iotaL_sb[:], in_=iotaL[:])
                ab1_sb = cp.tile([1, HALF], bf16)
                nc.sync.dma_start(out=ab1_sb[:], in_=ab1r[:])
                iotaR_sb = cp.tile([128, NG], f32)
                nc.sync.dma_start(out=iotaR_sb[:], in_=iotaRow[:])
                clamp_sb = cp.tile([128, 1], f32)
                nc.sync.dma_start(out=clamp_sb[:], in_=clampR[:])
                repM_sb = cp.tile([128, 128], f32)
                nc.sync.dma_start(out=repM_sb[:], in_=repMc[:])
                selMf_sb = cp.tile([128, BS], f32)
                nc.sync.dma_start(out=selMf_sb[:], in_=selMfc[:])
                maskC_sb = cp.tile([128, 24], f32)
                nc.sync.dma_start(out=maskC_sb[:], in_=maskCc[:])
                lmap_sb = cp.tile([24, NG * 128], f32)
                nc.sync.dma_start(out=lmap_sb[:], in_=lmapc[:])
                aw2_sb = cp.tile([128, HALF], f32)
                nc.sync.dma_start(out=aw2_sb[:], in_=aW2r[:])
                ab2_sb = cp.tile([128, 1], f32)
                nc.sync.dma_start(out=ab2_sb[:], in_=ab2[:])
                db1_sb = cp.tile([1, H], bf16)
                nc.sync.dma_start(out=db1_sb[:], in_=db1r[:])

                # big late-consumer loads issued now: their transfers run
                # during the start-barrier dead time alongside LSTM weights.
                dw2_sb = dw2p.tile([128, 8 * VS], bf16)
                for hf in range(2):
                    nc.sync.dma_start(out=dw2_sb[:, hf * 4 * VS:(hf + 1) * 4 * VS],
                                      in_=dW2T[:, hf * 4 * VS:(hf + 1) * 4 * VS])

                outT_sb = wk.tile([128, H], f32)      # final hidden, transposed
                outTb_sb = wk.tile([128, H], bf16)
                o2T = wk.tile([128, H], bf16)
                ago3 = dp.tile([H, 128], f32, name="ago3", addr_space="Shared")

                # ============================ LSTM ============================
                for l in range(NL):
                    g_ps = ps_g.tile([128, 512], f32, tag="g")
                    # h-half first: it has no dependence on the previous
                    # layer's AllGather, so it fills the AG gap on the PE.
                    for k in range(8):
                        nc.tensor.matmul(
                            out=g_ps[:],
                            lhsT=h0T_sb[:, (l * 8 + k) * 128:(l * 8 + k + 1) * 128],
                            rhs=w_sb[:, ((l * 2 + 1) * 8 + k) * 512:
                                     ((l * 2 + 1) * 8 + k + 1) * 512],
                            start=(k == 0), stop=False)
                    nc.tensor.matmul(out=g_ps[:], lhsT=ones_b[:],
                                     rhs=gb_sb[:, l * 512:(l + 1) * 512],
                                     start=False, stop=False)
                    for k in range(8):
                        nc.tensor.matmul(
                            out=g_ps[:],
                            lhsT=xT_sb[:, k * 128:(k + 1) * 128],
                            rhs=w_sb[:, ((l * 2) * 8 + k) * 512:
                                     ((l * 2) * 8 + k + 1) * 512],
                            start=False, stop=(k == 7))
                    # gates packed [i,f,o,g]: one sigmoid covers i/f/o
                    ifo = lwk.tile([128, 384], f32, tag="ifo")
                    nc.scalar.activation(out=ifo[:], in_=g_ps[:, 0:384],
                                         func=AF.Sigmoid)
                    g_t = lwk.tile([128, 128], f32, tag="g_t")
                    nc.scalar.activation(out=g_t[:], in_=g_ps[:, 384:512],
                                         func=AF.Tanh)
                    cnew = lwk.tile([128, 128], f32, tag="cnew")
                    nc.vector.tensor_mul(out=cnew[:], in0=ifo[:, 128:256],
                                         in1=c0_sb[:, l * HS:(l + 1) * HS])
                    ig = lwk.tile([128, 128], f32, tag="ig")
                    nc.vector.tensor_mul(out=ig[:], in0=ifo[:, 0:128], in1=g_t[:])
                    nc.vector.tensor_add(out=cnew[:], in0=cnew[:], in1=ig[:])
                    tc_t = lwk.tile([128, 128], f32, tag="tc_t")
                    nc.scalar.activation(out=tc_t[:], in_=cnew[:], func=AF.Tanh)
                    xf = lwk.tile([128, 128], f32, tag="xf")
                    nc.vector.tensor_mul(out=xf[:], in0=ifo[:, 256:384],
                                         in1=tc_t[:])

                    if l < NL - 1:
                        xb = lwk.tile([128, 128], bf16, tag="xb")
                        nc.vector.tensor_copy(out=xb[:], in_=xf[:])
                        tr_ps = ps_g.tile([128, 128], bf16, tag="tr")
                        nc.tensor.transpose(out=tr_ps[:], in_=xb[:], identity=idb[:])
                        xTs = lwk.tile([128, 128], bf16, tag="xTs")
                        nc.vector.tensor_copy(out=xTs[:], in_=tr_ps[:])
                        agi = dp.tile([128, 128], bf16, name=f"agi{l}", tag=f"agi{l}")
                        nc.sync.dma_start(out=agi[:], in_=xTs[:])
                        ago = dp.tile([H, 128], bf16, name=f"ago{l}", tag=f"ago{l}",
                                      addr_space="Shared")
                        nc.gpsimd.collective_compute(
                            "AllGather", OP.bypass, replica_groups=RG,
                            ins=[agi[:]], outs=[ago[:]])
                        xT_sb = lwk.tile([128, H], bf16, tag="xT", bufs=2)
                        nc.sync.dma_start(
                            out=xT_sb[:].rearrange("p (k b) -> p k b", b=128),
                            in_=ago[:].rearrange("(k p) b -> p k b", p=128))
                    else:
                        tr_ps = ps_g.tile([128, 128], f32, tag="trf")
                        nc.tensor.transpose(out=tr_ps[:], in_=xf[:], identity=idf[:])
                        agi3 = dp.tile([128, 128], f32, name="agi3")
                        xTf = lwk.tile([128, 128], f32, tag="xTf")
                        nc.vector.tensor_copy(out=xTf[:], in_=tr_ps[:])
                        nc.sync.dma_start(out=agi3[:], in_=xTf[:])
                        nc.gpsimd.collective_compute(
                            "AllGather", OP.bypass, replica_groups=RG,
                            ins=[agi3[:]], outs=[ago3[:]])

            with tc.tile_pool(name="dw1p", bufs=1) as dw1p:
                dw1_sb = dw1p.tile([128, 16 * H], bf16)
                for hf in range(2):
                    nc.sync.dma_start(out=dw1_sb[:, hf * 8 * H:(hf + 1) * 8 * H],
                                      in_=dW1T[:, hf * 8 * H:(hf + 1) * 8 * H])
                nc.sync.dma_start(
                    out=outT_sb[:].rearrange("p (k b) -> p k b", b=128),
                    in_=ago3[:].rearrange("(k p) b -> p k b", p=128))
                nc.vector.tensor_copy(out=outTb_sb[:], in_=outT_sb[:])
                lo_sb = wk.tile([128, H], bf16)
                nc.vector.tensor_tensor(out=lo_sb[:], in0=outT_sb[:],
                                        in1=outTb_sb[:], op=OP.subtract)

                # ============================ p-chain ============================
                with tc.tile_pool(name="att", bufs=1) as at, \
                     tc.tile_pool(name="ps_a", bufs=1, space="PSUM") as ps_a:
                    pt_ps = ps_a.tile([128, HALF], f32, tag="pt")
                    for k in range(8):
                        for lhsT, rhs in (
                                (outTb_sb, aw1h_sb), (outTb_sb, aw1l_sb),
                                (lo_sb, aw1h_sb)):
                            nc.tensor.matmul(
                                out=pt_ps[:],
                                lhsT=lhsT[:, k * 128:(k + 1) * 128],
                                rhs=rhs[:, k * HALF:(k + 1) * HALF],
                                start=(k == 0 and rhs is aw1h_sb
                                       and lhsT is outTb_sb),
                                stop=False)
                    nc.tensor.matmul(out=pt_ps[:], lhsT=ones_b[:], rhs=ab1_sb[:],
                                     start=False, stop=True)
                    pt = at.tile([128, HALF], f32)
                    nc.scalar.activation(out=pt[:], in_=pt_ps[:], func=AF.Tanh)
                    scr5 = at.tile([128, HALF], f32)
                    z = at.tile([128, 1], f32)
                    nc.vector.scalar_tensor_tensor(
                        out=scr5[:], in0=pt[:], scalar=1.0,
                        in1=aw2_sb[:], op0=OP.mult, op1=OP.mult,
                        accum_out=z[:])
                    sg = at.tile([128, 1], f32)
                    nc.scalar.activation(out=sg[:], in_=z[:], func=AF.Sigmoid,
                                         bias=ab2_sb[:])
                    s_f = at.tile([128, 1], f32)       # p - W  (pre-round)
                    nc.vector.tensor_scalar_mul(out=s_f[:], in0=sg[:],
                                                scalar1=float(S_val))
                    r_ = at.tile([128, 1], f32)
                    nc.vector.tensor_scalar_add(out=r_[:], in0=s_f[:], scalar1=0.5)
                    # floor(r_) robust to the f32->i32 cast rounding mode:
                    # f = cast(r_); if f > r_ then f -= 1
                    ti = at.tile([128, 1], i32)
                    nc.vector.tensor_copy(out=ti[:], in_=r_[:])
                    tf = at.tile([128, 1], f32)
                    nc.vector.tensor_copy(out=tf[:], in_=ti[:])
                    cond = at.tile([128, 1], f32)
                    nc.vector.tensor_tensor(out=cond[:], in0=tf[:], in1=r_[:],
                                            op=OP.is_gt)
                    stf = at.tile([128, 1], f32)       # start (rounded, float)
                    nc.vector.tensor_sub(out=stf[:], in0=tf[:], in1=cond[:])
                    d0 = at.tile([128, 1], f32)        # start - p  (= stf - s_f - W)
                    nc.vector.tensor_sub(out=d0[:], in0=stf[:], in1=s_f[:])
                    nc.vector.tensor_scalar_add(out=d0[:], in0=d0[:],
                                                scalar1=-float(W))

                    # ---- on-chip replicated out rows (this core's 16 b) ----
                    # rebuild out[b,h] via 8 full PE transposes of outT
                    # chunks, then orep = repM.T @ out replicates this
                    # core's 16 rows to the 128 (l,b) partition rows
                    # (repM is a per-core host constant, program is SPMD).
                    out_full = at.tile([128, H], f32)
                    for k in range(8):
                        tf_ps = ps_a.tile([128, 128], f32, tag="t16", bufs=2)
                        nc.tensor.transpose(
                            out=tf_ps[:],
                            in_=outT_sb[:, k * 128:(k + 1) * 128],
                            identity=idf[:])
                        nc.vector.tensor_copy(out=out_full[:, k * 128:(k + 1) * 128],
                                              in_=tf_ps[:])
                    orep_ps = ps_a.tile([128, H], f32, tag="orep")
                    for n in range(2):
                        nc.tensor.matmul(out=orep_ps[:, n * 512:(n + 1) * 512],
                                         lhsT=repM_sb[:],
                                         rhs=out_full[:, n * 512:(n + 1) * 512],
                                         start=True, stop=True)
                    orep = at.tile([128, H], f32)
                    nc.vector.tensor_copy(out=orep[:, 0:512], in_=orep_ps[:, 0:512])
                    nc.vector.tensor_copy(out=orep[:, 512:1024],
                                          in_=orep_ps[:, 512:1024])

                    # ---- cross-partition replication via 0/1 matmuls ----
                    pk2 = at.tile([128, 2], f32)
                    nc.vector.tensor_copy(out=pk2[:, 0:1], in_=stf[:])
                    nc.vector.tensor_copy(out=pk2[:, 1:2], in_=d0[:])
                    g16_ps = ps_a.tile([BS, 2], f32, tag="scr", bufs=1)
                    nc.tensor.matmul(out=g16_ps[:], lhsT=repM_sb[:, 0:BS],
                                     rhs=pk2[:], start=True, stop=True)
                    g16 = at.tile([BS, 2], f32)
                    nc.vector.tensor_copy(out=g16[:], in_=g16_ps[:])
                    stf16 = g16[:, 0:1]
                    d016 = g16[:, 1:2]
                    str_ps = ps_a.tile([128, 1], f32, tag="scr", bufs=1)
                    nc.tensor.matmul(out=str_ps[:], lhsT=repM_sb[:],
                                     rhs=stf[:], start=True, stop=True)
                    stf_rep = at.tile([128, 1], f32)
                    nc.vector.tensor_copy(out=stf_rep[:], in_=str_ps[:])

                    # ==================== gather ====================
                    sel = [at.tile([128, H], bf16, name=f"sel{g}", tag=f"sel{g}")
                           for g in range(NG)]
                    cnt = [128, 128, ROWS - 256]
                    idxs = []
                    for g in range(NG):
                        idxf = at.tile([128, 1], f32, tag=f"idxf{g}")
                        nc.vector.tensor_scalar_mul(out=idxf[:], in0=stf_rep[:],
                                                    scalar1=float(BS))
                        nc.vector.tensor_add(out=idxf[:], in0=idxf[:],
                                             in1=iotaR_sb[:, g:g + 1])
                        nc.vector.tensor_tensor(out=idxf[:], in0=idxf[:],
                                                in1=clamp_sb[:], op=OP.min)
                        idx = at.tile([128, 1], i32, tag=f"idx{g}")
                        nc.vector.tensor_copy(out=idx[:], in_=idxf[:])
                        idxs.append(idx)
                    for g in range(NG):
                        nc.gpsimd.indirect_dma_start(
                            out=sel[g][0:cnt[g], :], out_offset=None,
                            in_=encs[:],
                            in_offset=IOA(ap=idxs[g][0:cnt[g], :1], axis=0))

                    # ---- window masks + gaussian (overlap the gathers) ----
                    pos = at.tile([BS, L], f32)
                    nc.vector.tensor_scalar(out=pos[:], in0=iotaL_sb[:],
                                            scalar1=stf16, scalar2=None, op0=OP.add)
                    v1 = at.tile([BS, L], f32)
                    nc.vector.tensor_scalar(out=v1[:], in0=pos[:], scalar1=float(W),
                                            scalar2=None, op0=OP.is_ge)
                    v2 = at.tile([BS, L], f32)
                    nc.vector.tensor_scalar(out=v2[:], in0=pos[:],
                                            scalar1=float(S_val + W),
                                            scalar2=None, op0=OP.is_lt)
                    nc.vector.tensor_mul(out=v1[:], in0=v1[:], in1=v2[:])
                    dd = at.tile([BS, L], f32)
                    nc.vector.tensor_scalar(out=dd[:], in0=iotaL_sb[:],
                                            scalar1=d016, scalar2=None, op0=OP.add)
                    d2 = at.tile([BS, L], f32)
                    nc.vector.tensor_mul(out=d2[:], in0=dd[:], in1=dd[:])
                    gs = at.tile([BS, L], f32)
                    nc.scalar.activation(out=gs[:], in_=d2[:], func=AF.Exp,
                                         scale=-1.0 / (2.0 * STD2))

                    # ==================== scores ====================
                    sc_col = at.tile([128, NG], f32)
                    nc.vector.memset(sc_col[:], 0.0)
                    scrH = at.tile([128, H], f32)
                    for g in range(NG):
                        nc.vector.scalar_tensor_tensor(
                            out=scrH[0:cnt[g], :], in0=orep[0:cnt[g], :], scalar=1.0,
                            in1=sel[g][0:cnt[g], :], op0=OP.mult, op1=OP.mult,
                            accum_out=sc_col[0:cnt[g], g:g + 1])

                    # -------- [16, 21] softmax block --------
                    X = at.tile([128, 24], f32)
                    nc.vector.tensor_tensor(
                        out=X[:].rearrange("p (g li) -> p g li", g=NG),
                        in0=sc_col[:].unsqueeze(2).broadcast_to([128, NG, 8]),
                        in1=maskC_sb[:].rearrange("p (g li) -> p g li", g=NG),
                        op=OP.mult)
                    sc_ps = ps_a.tile([BS, 24], f32, tag="scr", bufs=1)
                    nc.tensor.matmul(out=sc_ps[:], lhsT=selMf_sb[:], rhs=X[:],
                                     start=True, stop=True)
                    sc24 = at.tile([BS, 24], f32)
                    nc.vector.tensor_copy(out=sc24[:], in_=sc_ps[:])
                    sc16 = sc24[:, 0:L]

                    sm = at.tile([BS, L], f32)
                    nc.vector.tensor_scalar_add(out=sm[:], in0=sc16, scalar1=-1e-12)
                    nc.vector.tensor_mul(out=sm[:], in0=sm[:], in1=v1[:])
                    nc.vector.tensor_scalar_add(out=sm[:], in0=sm[:], scalar1=1e-12)
                    mx = at.tile([BS, 1], f32)
                    nc.vector.tensor_reduce(out=mx[:], in_=sm[:],
                                            axis=mybir.AxisListType.X, op=OP.max)
                    nmx = at.tile([BS, 1], f32)
                    nc.vector.tensor_scalar_mul(out=nmx[:], in0=mx[:], scalar1=-1.0)
                    ex = at.tile([BS, L], f32)
                    se = at.tile([BS, 1], f32)
                    nc.scalar.activation(out=ex[:], in_=sm[:], func=AF.Exp,
                                         bias=nmx[:], accum_out=se[:])
                    ri = at.tile([BS, 1], f32)
                    nc.vector.reciprocal(out=ri[:], in_=se[:])
                    aa = at.tile([BS, L], f32)
                    nc.vector.tensor_scalar(out=aa[:], in0=ex[:], scalar1=ri[:],
                                            scalar2=None, op0=OP.mult)
                    nc.vector.tensor_mul(out=aa[:], in0=aa[:], in1=gs[:])
                    # relayout a -> a-weighted selector Sa via PE
                    aa24 = at.tile([BS, 24], f32)
                    nc.vector.memset(aa24[:], 0.0)
                    nc.vector.tensor_copy(out=aa24[:, 0:L], in_=aa[:])
                    aaT_ps = ps_a.tile([24, BS], f32, tag="scr", bufs=1)
                    nc.tensor.transpose(out=aaT_ps[:], in_=aa24[:],
                                        identity=idf[0:BS, 0:BS])
                    aaT = at.tile([24, BS], f32)
                    nc.vector.tensor_copy(out=aaT[:], in_=aaT_ps[:])

                    # ==================== ctx ====================
                    ctx_ps = ps_a.tile([BS, H], f32, tag="ctx")
                    for g in range(NG):
                        sa_ps = ps_a.tile([128, BS], f32, tag="scr", bufs=1)
                        nc.tensor.matmul(out=sa_ps[:],
                                         lhsT=lmap_sb[:, g * 128:(g + 1) * 128],
                                         rhs=aaT[:], start=True, stop=True)
                        sa = at.tile([128, BS], bf16, tag="sab", bufs=3)
                        nc.vector.tensor_mul(out=sa[:], in0=sa_ps[:],
                                             in1=selMf_sb[:])
                        for n in range(2):
                            nc.tensor.matmul(
                                out=ctx_ps[:, n * 512:(n + 1) * 512],
                                lhsT=sa[0:cnt[g], :],
                                rhs=sel[g][0:cnt[g], n * 512:(n + 1) * 512],
                                start=(g == 0), stop=(g == NG - 1))
                    ctxb = at.tile([BS, H], bf16)
                    nc.vector.tensor_copy(out=ctxb[:], in_=ctx_ps[:])
                    ctxi = dp.tile([BS, H], bf16, name="ctxi")
                    nc.sync.dma_start(out=ctxi[:], in_=ctxb[:])
                ctxo = dp.tile([B, H], bf16, name="ctxo", addr_space="Shared")
                nc.gpsimd.collective_compute(
                    "AllGather", OP.bypass, replica_groups=RG,
                    ins=[ctxi[:]], outs=[ctxo[:]])

                # ==================== o2 ====================
                o2b = wk.tile([128, H], bf16)
                with tc.tile_pool(name="o2w", bufs=1) as o2w, \
                     tc.tile_pool(name="ps_o2", bufs=1, space="PSUM") as ps_o2, \
                     tc.tile_pool(name="ps_tr", bufs=2, space="PSUM") as ps_tr:
                    o2_ps = ps_o2.tile([128, H], f32, tag="o2")
                    # out-half first: runs during the ctx AllGather
                    for k in range(8):
                        for n in range(2):
                            nc.tensor.matmul(
                                out=o2_ps[:, n * 512:(n + 1) * 512],
                                lhsT=outTb_sb[:, k * 128:(k + 1) * 128],
                                rhs=dw1_sb[:, (k + 8) * H + n * 512:
                                           (k + 8) * H + (n + 1) * 512],
                                start=(k == 0), stop=False)
                    for n in range(2):
                        nc.tensor.matmul(out=o2_ps[:, n * 512:(n + 1) * 512],
                                         lhsT=ones_b[:],
                                         rhs=db1_sb[:, n * 512:(n + 1) * 512],
                                         start=False, stop=False)
                    ctx_sb = o2w.tile([B, H], bf16)
                    nc.sync.dma_start(out=ctx_sb[:], in_=ctxo[:])
                    ctxT = o2w.tile([128, H], bf16)
                    for k in range(8):
                        trp = ps_tr.tile([128, 128], bf16, tag="tr2")
                        nc.tensor.transpose(out=trp[:],
                                            in_=ctx_sb[:, k * 128:(k + 1) * 128],
                                            identity=idb[:])
                        nc.vector.tensor_copy(out=ctxT[:, k * 128:(k + 1) * 128],
                                              in_=trp[:])
                    for k in range(8):
                        for n in range(2):
                            nc.tensor.matmul(
                                out=o2_ps[:, n * 512:(n + 1) * 512],
                                lhsT=ctxT[:, k * 128:(k + 1) * 128],
                                rhs=dw1_sb[:, k * H + n * 512:k * H + (n + 1) * 512],
                                start=False, stop=(k == 7))
                    nc.scalar.activation(out=o2b[:], in_=o2_ps[:], func=AF.Tanh)
                    for k in range(8):
                        trp = ps_tr.tile([128, 128], bf16, tag="tr2")
                        nc.tensor.transpose(out=trp[:],
                                            in_=o2b[:, k * 128:(k + 1) * 128],
                                            identity=idb[:])
                        nc.vector.tensor_copy(out=o2T[:, k * 128:(k + 1) * 128],
                                              in_=trp[:])

            # ==================== vocab ====================
            # two 2048-col halves; within a half, k outer / n inner so each
            # lhsT streak shares its weight load and 4 PSUM banks accumulate.
            with tc.tile_pool(name="ps_y", bufs=2, space="PSUM") as ps_y, \
                 tc.tile_pool(name="ysb", bufs=2) as ysb:
                for h2 in range(2):
                    hw = 2048 if h2 == 0 else VS - 2048   # 2048 | 1952
                    y_ps = ps_y.tile([128, 2048], f32, tag="y")
                    for k in range(8):
                        for n in range(4):
                            c0c = h2 * 2048 + n * 512
                            cw = min(512, VS - c0c)
                            nc.tensor.matmul(
                                out=y_ps[:, n * 512:n * 512 + cw],
                                lhsT=o2T[:, k * 128:(k + 1) * 128],
                                rhs=dw2_sb[:, k * VS + c0c:k * VS + c0c + cw],
                                start=(k == 0), stop=(k == 7))
                    y_sb = ysb.tile([128, 2048], f32, tag="ysb")
                    nc.vector.tensor_copy(out=y_sb[:, 0:hw], in_=y_ps[:, 0:hw])
                    nc.sync.dma_start(out=yout[:, h2 * 2048:h2 * 2048 + hw],
                                      in_=y_sb[:, 0:hw])

    nc.compile()
    return nc


def _prep_inputs(inputs):
    """Host-side layout: returns list of per-core in_maps."""
    import ml_dtypes
    bf16 = ml_dtypes.bfloat16

    enc = np.asarray(inputs["encoder_output"], np.float32)      # [T, B, H]
    h0 = np.asarray(inputs["h0"], np.float32)
    c0 = np.asarray(inputs["c0"], np.float32)
    emb = np.asarray(inputs["emb"], np.float32)
    Wih = np.asarray(inputs["Wih"], np.float32)
    Whh = np.asarray(inputs["Whh"], np.float32)
    bih = np.asarray(inputs["bih"], np.float32)
    bhh = np.asarray(inputs["bhh"], np.float32)
    aW1 = np.asarray(inputs["aW1"], np.float32)
    aW2 = np.asarray(inputs["aW2"], np.float32)
    ab2 = np.asarray(inputs["ab2"], np.float32)
    dW1 = np.asarray(inputs["dW1"], np.float32)
    db1 = np.asarray(inputs["db1"], np.float32)
    dW2 = np.asarray(inputs["dW2"], np.float32)
    word = np.asarray(inputs["word"]).astype(np.int64)

    def shuf(a, k, c):
        # [k*128, c] -> [128, k*c]: per-partition contiguous DMA lines
        return np.ascontiguousarray(
            a.reshape(k, 128, c).transpose(1, 0, 2).reshape(128, k * c))

    x0 = emb[word[0]]                                            # [B, H]
    x0T = shuf(np.ascontiguousarray(x0.T).astype(bf16), 8, B)
    h0T = shuf(np.ascontiguousarray(
        h0.transpose(0, 2, 1)).reshape(NL * H, B).astype(bf16), NL * 8, B)

    ident_b = np.eye(128, dtype=np.float32).astype(bf16)
    ident_f = np.eye(128, dtype=np.float32)
    iotaL = np.tile(np.arange(L, dtype=np.float32).reshape(1, L), (BS, 1))
    iotaRow = np.zeros((128, NG), np.float32)
    for g in range(NG):
        for p in range(128):
            r = g * 128 + p
            iotaRow[p, g] = float(r if r < ROWS else 0)
    clampR = ((T - 1) * BS + (np.arange(128) % BS)).astype(np.float32).reshape(128, 1)
    selMf = np.zeros((128, BS), np.float32)
    for p in range(128):
        selMf[p, p % BS] = 1.0
    maskC = np.zeros((128, 24), np.float32)
    for p in range(128):
        for c in range(24):
            if p // BS == c % 8:
                maskC[p, c] = 1.0
    lmap = np.zeros((24, NG * 128), np.float32)
    for g in range(NG):
        for row in range(128):
            lmap[g * 8 + row // BS, g * 128 + row] = 1.0

    dW1T = shuf(np.ascontiguousarray(dW1.T).astype(bf16), 16, H)
    aW1T = np.ascontiguousarray(aW1.T)                           # [H, HALF] f32
    aW1Th32 = aW1T.astype(bf16).astype(np.float32)
    aW1Th = shuf(aW1Th32.astype(bf16), 8, HALF)
    aW1Tl = shuf((aW1T - aW1Th32).astype(bf16), 8, HALF)
    aW2r = np.tile(aW2.reshape(1, HALF), (128, 1)).astype(np.float32)
    ab2r = np.tile(ab2.reshape(1, 1), (128, 1)).astype(np.float32)
    db1r = db1.reshape(1, H).astype(bf16)
    ab1r = np.asarray(inputs["ab1"], np.float32).reshape(1, HALF).astype(bf16)

    GATE_ORDER = [0, 1, 3, 2]   # [i, f, o, g] so one sigmoid covers i/f/o
    in_maps = []
    for r in range(NC):
        hs = slice(r * HS, (r + 1) * HS)
        rows = np.concatenate([np.arange(g * H + r * HS, g * H + (r + 1) * HS)
                               for g in GATE_ORDER])
        wT_l = []
        gb = np.zeros((NL, 512), np.float32)
        for l in range(NL):
            wT_l.append(np.ascontiguousarray(Wih[l][rows, :].T))  # [H, 512]
            wT_l.append(np.ascontiguousarray(Whh[l][rows, :].T))
            gb[l] = bih[l][rows] + bhh[l][rows]
        wT = shuf(np.concatenate(wT_l, axis=0).astype(bf16), NL * 2 * 8, 512)
        c0s = np.ascontiguousarray(
            np.stack([c0[l][:, hs] for l in range(NL)], axis=1).reshape(B, NL * HS))
        bs = slice(r * BS, (r + 1) * BS)
        encs = np.ascontiguousarray(enc[:, bs, :]).reshape(T * BS, H).astype(bf16)
        vs = slice(r * VS, (r + 1) * VS)
        dW2T = shuf(np.ascontiguousarray(dW2[vs, :].T).astype(bf16), 8, VS)
        repM = np.zeros((128, 128), np.float32)
        for m in range(128):
            repM[r * BS + (m % BS), m] = 1.0
        in_maps.append({
            "x0T": np.ascontiguousarray(x0T),
            "h0T": h0T, "c0s": c0s, "wT": wT,
            "gbias": gb.reshape(1, NL * 512).astype(bf16),
            "aW1Th": aW1Th, "aW1Tl": aW1Tl, "aW2r": aW2r, "ab2": ab2r,
            "dW1T": dW1T, "db1r": db1r, "ab1r": ab1r, "dW2T": dW2T,
            "encs": encs, "identb": ident_b, "identf": ident_f,
            "iotaL": iotaL, "iotaRow": iotaRow, "clampR": clampR,
            "repMc": repM, "selMfc": selMf,
            "maskCc": maskC, "lmapc": lmap,
        })
    return in_maps


def kernel(**inputs):
    from concourse import bass_utils
    S_val = float(np.asarray(inputs["S"]))
    key = ("mod", S_val)
    if key not in _CACHE:
        _CACHE[key] = _build(S_val)
    nc = _CACHE[key]
    in_maps = _prep_inputs(inputs)
    res = bass_utils.run_bass_kernel_spmd(nc, in_maps, core_ids=list(range(NC)))
    y = np.concatenate([res.results[r]["yout"] for r in range(NC)], axis=1)
    y = y + np.asarray(inputs["db2"], np.float32).reshape(1, V)
    return y.reshape(1, B, V).astype(np.float32)
